# revision 1
# baseline (speedup 1.0000x reference)
"""Trainium2 Bass kernel for nn_Block_77318001263203 (dense transformer block).

Distribution over 8 NeuronCores: data-parallel over batch (2 groups of 4
cores) x tensor-parallel over heads (4 heads/core) for attention+proj,
4-way-chunked ReduceScatter of the proj partials over each 4-core group
(pipelined with attention compute; each chunk hands every rank one
128-token block, so rank r owns the strided token set
{512*ck + 128*r + j}), then token-parallel FFN with full (replicated)
FFN weights — no second collective. All matmuls run as float32r (full PE
rate, ~2e-4 rel err).

kernel(**inputs) takes the FULL inputs from setup_inputs() and returns the
FULL [2, 2048, 1024] output.
"""

import numpy as np

import concourse.bass as bass
import concourse.mybir as mybir
import concourse.tile as tile
from concourse import bacc
from concourse.bass_utils import run_bass_kernel_spmd
from concourse.masks import make_identity

# problem dims (hardcoded per the harness contract)
B, S, D = 2, 2048, 1024
H, HS, F = 16, 64, 4096
EPS = 1e-5
P = 128
NCORES = 8
TP = 4  # cores per batch group
HPC = H // TP  # heads per core = 4
SL = S // TP  # tokens owned per core = 512 (4 strided blocks of 128)
QT = 512  # query tile
KB = 128  # key block
NCK = 4  # reduce-scatter chunks
NEG = -1.0e9  # additive causal mask (exp underflows to exactly 0)

f32 = mybir.dt.float32
f32r = mybir.dt.float32r

REPLICA_GROUPS = [[0, 1, 2, 3], [4, 5, 6, 7]]


def _bcast_row_ap(t, row, width):
    """DMA-source AP broadcasting row `row` of DRAM tensor t to 128 partitions."""
    return bass.AP(tensor=t, offset=row * width, ap=[[0, P], [1, width]])


def build_bass():
    nc = bacc.Bacc("TRN2", target_bir_lowering=False, debug=False, num_devices=NCORES)

    xT = nc.dram_tensor("xT", [D, S], f32, kind="ExternalInput").ap()
    xs = nc.dram_tensor("xs", [SL, D], f32, kind="ExternalInput").ap()
    wq2 = nc.dram_tensor("wq2", [D, HPC * HS], f32, kind="ExternalInput").ap()
    wk2 = nc.dram_tensor("wk2", [D, HPC * HS], f32, kind="ExternalInput").ap()
    wv4 = nc.dram_tensor("wv4", [D, HPC * HS], f32, kind="ExternalInput").ap()
    wp = nc.dram_tensor("wp", [HPC * HS, D], f32, kind="ExternalInput").ap()
    w1 = nc.dram_tensor("w1", [D, F], f32, kind="ExternalInput").ap()
    w2 = nc.dram_tensor("w2", [F, D], f32, kind="ExternalInput").ap()
    cvec = nc.dram_tensor("cvec", [6, D], f32, kind="ExternalInput").ap()
    b1d = nc.dram_tensor("b1d", [F], f32, kind="ExternalInput").ap()
    out = nc.dram_tensor("out", [SL, D], f32, kind="ExternalOutput").ap()

    # per-chunk collective bounce buffers (separate tensors -> precise deps)
    rs_in = [nc.dram_tensor(f"rs_in{c}", [S // NCK, D], f32) for c in range(NCK)]
    rs_out = [nc.dram_tensor(f"rs_out{c}", [P, D], f32) for c in range(NCK)]

    # additive causal triangle mask for the 128x128 diagonal block:
    # keep (0) where t <= q, NEG where t > q
    m_np = np.where(
        np.arange(KB)[:, None] <= np.arange(KB)[None, :], 0.0, NEG
    ).astype(np.float32)
    masks_dram = nc.inline_tensor(m_np, name="causal_mask")

    with tile.TileContext(nc) as tc:
        with tc.tile_pool(name="const", bufs=1) as constp:
            ident_f = constp.tile([P, P], f32)
            make_identity(nc, ident_f)
            ident = constp.tile([P, P], f32r)
            nc.vector.tensor_copy(ident, ident_f)
            eps_t = constp.tile([P, 1], f32)
            nc.vector.memset(eps_t, EPS)
            b1_sb = constp.tile([P, F // P], f32)
            nc.sync.dma_start(b1_sb, b1d.rearrange("(ko p) -> p ko", p=P))
            g2b = constp.tile([P, D], f32)
            nc.gpsimd.dma_start(g2b, _bcast_row_ap(cvec.tensor, 3, D))
            be2b = constp.tile([P, D], f32)
            nc.gpsimd.dma_start(be2b, _bcast_row_ap(cvec.tensor, 4, D))
            b2b = constp.tile([P, D], f32)
            nc.gpsimd.dma_start(b2b, _bcast_row_ap(cvec.tensor, 5, D))

            # ---------------- Phase A: QKV + attention + proj + chunked RS ---
            with (
                tc.tile_pool(name="wqkvp", bufs=1) as wqkvp,
                tc.tile_pool(name="qkvo", bufs=1) as qkvo,
                tc.tile_pool(name="xrp", bufs=2) as xrp,
                tc.tile_pool(name="smallp", bufs=4) as smallp,
                tc.tile_pool(name="projp", bufs=3) as projp,
            ):
                # QKV projection weights first (on the critical path)
                wq_sb = wqkvp.tile([P, D // P, HPC * HS], f32r, tag="wq")
                nc.sync.dma_start(
                    wq_sb, wq2.rearrange("(ko p) m -> p ko m", p=P).bitcast(f32r)
                )
                wk_sb = wqkvp.tile([P, D // P, HPC * HS], f32r, tag="wk")
                nc.sync.dma_start(
                    wk_sb, wk2.rearrange("(ko p) m -> p ko m", p=P).bitcast(f32r)
                )
                wv_sb = wqkvp.tile([P, D // P, HPC * HS], f32r, tag="wv")
                nc.sync.dma_start(
                    wv_sb, wv4.rearrange("(ko p) m -> p ko m", p=P).bitcast(f32r)
                )

                # outputs of QKV: qT/kT per head pair, v (+ones col) per head
                q2T = qkvo.tile([P, 2, S], f32r, tag="q2T")
                k2T = qkvo.tile([P, 2, S], f32r, tag="k2T")
                v4e = qkvo.tile([P, S // P, HPC * (HS + 1)], f32r, tag="v4e")
                attnT = qkvo.tile([P, 2, S], f32r, tag="attnT")
                ones4 = qkvo.tile([P, HPC, 1], f32, tag="ones4")
                nc.vector.memset(ones4, 1.0)

                with tc.tile_pool(name="ps_qkv", bufs=4, space="PSUM") as psq:
                    for tt in range(S // QT):
                        xr = xrp.tile([P, D // P, QT], f32r, tag="xr")
                        nc.sync.dma_start(
                            xr,
                            xT[:, tt * QT : (tt + 1) * QT]
                            .rearrange("(ko p) m -> p ko m", p=P)
                            .bitcast(f32r),
                        )
                        for hp in range(2):
                            qps = psq.tile([P, QT], f32, tag="qk")
                            for ko in range(D // P):
                                nc.tensor.matmul(
                                    qps,
                                    wq_sb[:, ko, hp * P : (hp + 1) * P],
                                    xr[:, ko, :],
                                    start=(ko == 0),
                                    stop=(ko == D // P - 1),
                                )
                            nc.vector.tensor_copy(
                                q2T[:, hp, tt * QT : (tt + 1) * QT], qps
                            )
                            kps = psq.tile([P, QT], f32, tag="qk")
                            for ko in range(D // P):
                                nc.tensor.matmul(
                                    kps,
                                    wk_sb[:, ko, hp * P : (hp + 1) * P],
                                    xr[:, ko, :],
                                    start=(ko == 0),
                                    stop=(ko == D // P - 1),
                                )
                            nc.vector.tensor_copy(
                                k2T[:, hp, tt * QT : (tt + 1) * QT], kps
                            )
                        for mt in range(QT // P):
                            vps = psq.tile([P, HPC * HS], f32, tag="v")
                            for ko in range(D // P):
                                nc.tensor.matmul(
                                    vps,
                                    xr[:, ko, mt * P : (mt + 1) * P],
                                    wv_sb[:, ko, :],
                                    start=(ko == 0),
                                    stop=(ko == D // P - 1),
                                )
                            idx = tt * (QT // P) + mt
                            vv = v4e[:, idx, :].rearrange("p (h e) -> p h e", e=HS + 1)
                            nc.vector.tensor_copy(
                                vv[:, :, 0:HS],
                                vps.rearrange("p (h e) -> p h e", e=HS),
                            )
                            nc.vector.tensor_copy(vv[:, :, HS : HS + 1], ones4)

                # proj weights + masks: needed later, keep off the startup path
                wp_sb = wqkvp.tile([P, (HPC * HS) // P, D], f32r, tag="wp")
                nc.sync.dma_start(
                    wp_sb, wp.rearrange("(ko p) n -> p ko n", p=P).bitcast(f32r)
                )
                masks_sb = wqkvp.tile([P, KB], f32, tag="masks")
                nc.sync.dma_start(masks_sb, masks_dram.ap())

                with (
                    tc.tile_pool(name="ps_sc", bufs=2, space="PSUM") as pssc,
                    tc.tile_pool(name="ps_at", bufs=3, space="PSUM") as psat,
                    tc.tile_pool(name="ps_pr", bufs=1, space="PSUM") as pspr,
                ):
                    for qt in range(S // QT):
                        nkb = 4 * qt + 4
                        qsl = slice(qt * QT, (qt + 1) * QT)
                        for hp in range(2):
                            apair = psat.tile([HS + 1, QT], f32, tag="at")
                            apodd = psat.tile([HS + 1, QT], f32, tag="at")
                            for kb in range(nkb):
                                ksl = slice(kb * KB, (kb + 1) * KB)
                                moff = kb - 4 * qt
                                # diagonal blocks: columns [0, KB*moff) are fully
                                # masked -> skip them entirely
                                q0 = KB * moff if moff > 0 else 0
                                qr = slice(qt * QT + q0, (qt + 1) * QT)
                                sp = pssc.tile([P, 2, QT], f32, tag="sc")
                                nc.tensor.matmul(
                                    sp[:, 0, q0:],
                                    k2T[0:HS, hp, ksl],
                                    q2T[0:HS, hp, qr],
                                    start=True,
                                    stop=True,
                                    tile_position=(0, 0),
                                )
                                nc.tensor.matmul(
                                    sp[:, 1, q0:],
                                    k2T[HS : 2 * HS, hp, ksl],
                                    q2T[HS : 2 * HS, hp, qr],
                                    start=True,
                                    stop=True,
                                    tile_position=(64, 0),
                                )
                                if moff >= 0:
                                    dia = slice(KB * moff, KB * (moff + 1))
                                    nc.vector.tensor_add(
                                        sp[:, :, dia],
                                        sp[:, :, dia],
                                        masks_sb[:, None, :].to_broadcast(
                                            (P, 2, KB)
                                        ),
                                    )
                                ee = smallp.tile([P, 2, QT], f32r, tag="ee")
                                nc.scalar.activation(
                                    out=ee[:, :, q0:],
                                    in_=sp[:, :, q0:],
                                    func=mybir.ActivationFunctionType.Exp,
                                    scale=float(HS) ** -0.5,
                                )
                                he = (2 * hp) * (HS + 1)
                                ho = (2 * hp + 1) * (HS + 1)
                                nc.tensor.matmul(
                                    apair[:, q0:],
                                    v4e[:, kb, he : he + HS + 1],
                                    ee[:, 0, q0:],
                                    start=(kb == 0),
                                    stop=(kb == nkb - 1),
                                )
                                nc.tensor.matmul(
                                    apodd[:, q0:],
                                    v4e[:, kb, ho : ho + HS + 1],
                                    ee[:, 1, q0:],
                                    start=(kb == 0),
                                    stop=(kb == nkb - 1),
                                )
                            # quick PSUM->SBUF copy (frees accumulators), then
                            # normalize in SBUF off the PE critical path
                            for par, aps in ((0, apair), (1, apodd)):
                                ua = smallp.tile([HS + 1, QT], f32, tag="ua")
                                nc.vector.tensor_copy(ua, aps)
                                rec = smallp.tile([1, QT], f32, tag="rec")
                                nc.vector.reciprocal(rec, ua[HS : HS + 1, :])
                                bc = smallp.tile([HS, QT], f32, tag="bc")
                                nc.gpsimd.partition_broadcast(bc, rec)
                                nc.vector.tensor_mul(
                                    attnT[par * HS : (par + 1) * HS, hp, qsl],
                                    ua[0:HS, :],
                                    bc,
                                )
                        # proj for this qt's 4 token tiles, then RS chunk qt
                        for mtl in range(4):
                            mt = 4 * qt + mtl
                            prj = projp.tile([P, D], f32, tag="prj")
                            for nh in range(D // QT):
                                pps = pspr.tile([P, QT], f32, tag="pr")
                                for ko in range(2):
                                    nc.tensor.matmul(
                                        pps,
                                        attnT[:, ko, mt * P : (mt + 1) * P],
                                        wp_sb[:, ko, nh * QT : (nh + 1) * QT],
                                        start=(ko == 0),
                                        stop=(ko == 1),
                                    )
                                nc.vector.tensor_copy(
                                    prj[:, nh * QT : (nh + 1) * QT], pps
                                )
                            nc.sync.dma_start(
                                rs_in[qt].ap()[mtl * P : (mtl + 1) * P, :], prj
                            )
                        nc.gpsimd.collective_compute(
                            "ReduceScatter",
                            mybir.AluOpType.add,
                            replica_groups=REPLICA_GROUPS,
                            ins=[rs_in[qt].ap().opt()],
                            outs=[rs_out[qt].ap().opt()],
                        )

            # ---------------- Phase B: LN1 + FFN + LN2 ----------------
            with tc.tile_pool(name="ffn_keep", bufs=1) as keep:
                x1r = keep.tile([P, SL // P, D], f32r, tag="x1r")
                hT = keep.tile([P, F // P, SL], f32r, tag="hT")
                x1tp_cm = tc.tile_pool(name="x1tp", bufs=1)
                x1tp = x1tp_cm.__enter__()
                x1T = x1tp.tile([P, D // P, SL], f32r, tag="x1T")

                with (
                    tc.tile_pool(name="ln1p", bufs=2) as ln1p,
                    tc.tile_pool(name="ln1c", bufs=1) as ln1c,
                    tc.tile_pool(name="ps_tr", bufs=2, space="PSUM") as pstr,
                ):
                    g1b = ln1c.tile([P, D], f32, tag="g1b")
                    nc.gpsimd.dma_start(g1b, _bcast_row_ap(cvec.tensor, 1, D))
                    be1b = ln1c.tile([P, D], f32, tag="be1b")
                    nc.gpsimd.dma_start(be1b, _bcast_row_ap(cvec.tensor, 2, D))
                    bpb = ln1c.tile([P, D], f32, tag="bpb")
                    nc.gpsimd.dma_start(bpb, _bcast_row_ap(cvec.tensor, 0, D))

                    for st in range(SL // P):
                        y = ln1p.tile([P, D], f32, tag="y")
                        nc.sync.dma_start(y, rs_out[st].ap())
                        xst = ln1p.tile([P, D], f32, tag="xst")
                        nc.sync.dma_start(xst, xs[st * P : (st + 1) * P, :])
                        nc.vector.tensor_add(y, y, xst)
                        nc.vector.tensor_add(y, y, bpb)
                        stats = ln1p.tile([P, 2, 6], f32, tag="stats")
                        yv = y.rearrange("p (s d) -> p s d", s=2)
                        nc.vector.bn_stats(out=stats[:, 0, :], in_=yv[:, 0, :])
                        nc.vector.bn_stats(out=stats[:, 1, :], in_=yv[:, 1, :])
                        mv = ln1p.tile([P, 2], f32, tag="mv")
                        nc.vector.bn_aggr(out=mv, in_=stats)
                        rstd = ln1p.tile([P, 1], f32, tag="rstd")
                        nc.scalar.activation(
                            out=rstd,
                            in_=mv[:, 1:2],
                            func=mybir.ActivationFunctionType.Sqrt,
                            bias=eps_t,
                            scale=1.0,
                        )
                        nc.vector.reciprocal(rstd, rstd)
                        tmp = ln1p.tile([P, D], f32, tag="tmp")
                        nc.vector.tensor_scalar(
                            out=tmp,
                            in0=y,
                            scalar1=mv[:, 0:1],
                            scalar2=rstd,
                            op0=mybir.AluOpType.subtract,
                            op1=mybir.AluOpType.mult,
                        )
                        nc.vector.tensor_mul(tmp, tmp, g1b)
                        nc.vector.tensor_add(x1r[:, st, :], tmp, be1b)
                        # transpose this token tile into x1T
                        for dk in range(D // P):
                            tp = pstr.tile([P, P], f32r, tag="tp")
                            nc.tensor.transpose(
                                tp, x1r[:, st, dk * P : (dk + 1) * P], ident
                            )
                            nc.vector.tensor_copy(
                                x1T[:, dk, st * P : (st + 1) * P], tp
                            )

                # FFN first matmul: hT[f, tok] = w1.T @ x1T, relu(+b1) fused
                with (
                    tc.tile_pool(name="w1p", bufs=3) as w1p,
                    tc.tile_pool(name="ps_h", bufs=2, space="PSUM") as psh,
                ):
                    for ft in range(F // P):
                        w1t = w1p.tile([P, D // P, P], f32r, tag="w1t")
                        nc.sync.dma_start(
                            w1t,
                            w1[:, ft * P : (ft + 1) * P]
                            .rearrange("(ko p) m -> p ko m", p=P)
                            .bitcast(f32r),
                        )
                        hps = psh.tile([P, SL], f32, tag="h")
                        for ko in range(D // P):
                            nc.tensor.matmul(
                                hps,
                                w1t[:, ko, :],
                                x1T[:, ko, :],
                                start=(ko == 0),
                                stop=(ko == D // P - 1),
                            )
                        nc.scalar.activation(
                            out=hT[:, ft, :],
                            in_=hps,
                            func=mybir.ActivationFunctionType.Relu,
                            bias=b1_sb[:, ft : ft + 1],
                            scale=1.0,
                        )
                x1tp_cm.__exit__(None, None, None)

                # FFN second matmul (directly in [tok, d] layout) + residual + LN2
                with (
                    tc.tile_pool(name="w2p", bufs=2) as w2p,
                    tc.tile_pool(name="zp", bufs=1) as zp,
                    tc.tile_pool(name="ln2p", bufs=2) as ln2p,
                    tc.tile_pool(name="ps_y", bufs=4, space="PSUM") as psy,
                ):
                    NQ = 512  # d-half width
                    NKO = F // (2 * P)  # 16 k-subtiles per w2 tile
                    z = zp.tile([P, SL // P, D], f32, tag="z")
                    for dtq in range(D // NQ):
                        ypss = [
                            psy.tile([P, NQ], f32, tag="yq", name=f"yq_{dtq}_{i}")
                            for i in range(SL // P)
                        ]
                        for kh in range(2):
                            w2t = w2p.tile([P, NKO, NQ], f32r, tag="w2t")
                            nc.sync.dma_start(
                                w2t,
                                w2[
                                    kh * (F // 2) : (kh + 1) * (F // 2),
                                    dtq * NQ : (dtq + 1) * NQ,
                                ]
                                .rearrange("(ko p) n -> p ko n", p=P)
                                .bitcast(f32r),
                            )
                            for mt in range(SL // P):
                                for ko in range(NKO):
                                    nc.tensor.matmul(
                                        ypss[mt],
                                        hT[:, kh * NKO + ko, mt * P : (mt + 1) * P],
                                        w2t[:, ko, :],
                                        start=(kh == 0 and ko == 0),
                                        stop=(kh == 1 and ko == NKO - 1),
                                    )
                        dsl = slice(dtq * NQ, (dtq + 1) * NQ)
                        for mt in range(SL // P):
                            nc.vector.tensor_add(
                                z[:, mt, dsl], ypss[mt], x1r[:, mt, dsl]
                            )
                    # z += b2, then LN2 -> out
                    for mt in range(SL // P):
                        zm = z[:, mt, :]
                        nc.vector.tensor_add(zm, zm, b2b)
                        stats = ln2p.tile([P, 2, 6], f32, tag="stats2")
                        zv = zm.rearrange("p (s d) -> p s d", s=2)
                        nc.vector.bn_stats(out=stats[:, 0, :], in_=zv[:, 0, :])
                        nc.vector.bn_stats(out=stats[:, 1, :], in_=zv[:, 1, :])
                        mv = ln2p.tile([P, 2], f32, tag="mv2")
                        nc.vector.bn_aggr(out=mv, in_=stats)
                        rstd = ln2p.tile([P, 1], f32, tag="rstd2")
                        nc.scalar.activation(
                            out=rstd,
                            in_=mv[:, 1:2],
                            func=mybir.ActivationFunctionType.Sqrt,
                            bias=eps_t,
                            scale=1.0,
                        )
                        nc.vector.reciprocal(rstd, rstd)
                        o = ln2p.tile([P, D], f32, tag="o")
                        nc.vector.tensor_scalar(
                            out=o,
                            in0=zm,
                            scalar1=mv[:, 0:1],
                            scalar2=rstd,
                            op0=mybir.AluOpType.subtract,
                            op1=mybir.AluOpType.mult,
                        )
                        nc.vector.tensor_mul(o, o, g2b)
                        nc.vector.tensor_add(o, o, be2b)
                        nc.sync.dma_start(out[mt * P : (mt + 1) * P, :], o)

    nc.compile()
    return nc


_NC_CACHE = []


def _get_nc():
    if not _NC_CACHE:
        _NC_CACHE.append(build_bass())
    return _NC_CACHE[0]


def _token_blocks(r):
    """Global token rows (within a batch element) owned by rank r, as NCK
    blocks of 128: block ck covers rows [512*ck + 128*r, 512*ck + 128*r + 128)."""
    return [slice(QT * ck + P * r, QT * ck + P * r + P) for ck in range(NCK)]


def make_in_maps(x, wq, wk, wv, w_proj, b_proj, w1, b1, w2, b2, g1, be1, g2, be2):
    x = np.asarray(x, dtype=np.float32)
    cat = lambda w, h0: np.ascontiguousarray(
        np.concatenate(
            [np.asarray(w[h0 + i], dtype=np.float32) for i in range(HPC)], axis=1
        )
    )
    cvec_rows = [b_proj, g1, be1, g2, be2, b2]
    cvec = np.ascontiguousarray(
        np.stack([np.asarray(v, dtype=np.float32) for v in cvec_rows])
    )
    w1c = np.ascontiguousarray(np.asarray(w1, dtype=np.float32))
    w2c = np.ascontiguousarray(np.asarray(w2, dtype=np.float32))
    b1c = np.ascontiguousarray(np.asarray(b1, dtype=np.float32))
    wpc = np.ascontiguousarray(np.asarray(w_proj, dtype=np.float32))
    xTs = [np.ascontiguousarray(x[g].T) for g in range(B)]
    in_maps = []
    for c in range(NCORES):
        g, r = divmod(c, TP)
        h0 = HPC * r
        xs_blocks = np.concatenate([x[g, blk] for blk in _token_blocks(r)], axis=0)
        in_maps.append(
            {
                "xT": xTs[g],
                "xs": np.ascontiguousarray(xs_blocks),
                "wq2": cat(wq, h0),
                "wk2": cat(wk, h0),
                "wv4": cat(wv, h0),
                "wp": np.ascontiguousarray(wpc[HPC * HS * r : HPC * HS * (r + 1)]),
                "w1": w1c,
                "w2": w2c,
                "cvec": cvec,
                "b1d": b1c,
            }
        )
    return in_maps


def assemble(results):
    full = np.empty((B, S, D), dtype=np.float32)
    for c in range(NCORES):
        g, r = divmod(c, TP)
        o = results[c]["out"]
        for ck, blk in enumerate(_token_blocks(r)):
            full[g, blk] = o[ck * P : (ck + 1) * P]
    return full


def kernel(**inputs):
    nc = _get_nc()
    in_maps = make_in_maps(**inputs)
    res = run_bass_kernel_spmd(nc, in_maps, core_ids=list(range(NCORES)))
    return assemble(res.results)



# revision 18
# speedup vs baseline: 1.0401x; 1.0401x over previous
"""Trainium2 Bass kernel for nn_Block_77318001263203 (dense transformer block).

Distribution over 8 NeuronCores: data-parallel over batch (2 groups of 4
cores) x tensor-parallel over heads (4 heads/core) for attention+proj,
4-way-chunked bf16 ReduceScatter of the proj partials over each 4-core
group (each chunk hands every rank one 128-token block, so rank r owns
the strided token set {512*ck + 128*r + j}), then token-parallel FFN with
replicated FFN weights — no second collective.

v2 vs the original: all matmuls run in bf16 (weights cast host-side,
activations cast on device; fp32 PSUM accumulate), halving HBM/collective
bytes and PE power. The attention score matmuls drop the float32r
tile_position packing (that packing faults on hardware with bf16) and run
as plain K=64 matmuls. Phase B (LN1/transpose/FFN/LN2) is pipelined into
the attention tail by emission order: LN1 of token tiles 0-1 is emitted
behind ReduceScatter chunk 2, FFN half 0 behind chunk 3, keeping the PE
busy through the collectives and the HAM activity window warm. b_proj is
pre-added into the xs residual host-side.

kernel(**inputs) takes the FULL inputs from setup_inputs() and returns the
FULL [2, 2048, 1024] float32 output.
"""

import numpy as np
import ml_dtypes

import concourse.bass as bass
import concourse.mybir as mybir
import concourse.tile as tile
from concourse import bacc
from concourse.bass_utils import run_bass_kernel_spmd
from concourse.masks import make_identity

# problem dims (hardcoded per the harness contract)
B, S, D = 2, 2048, 1024
H, HS, F = 16, 64, 4096
EPS = 1e-5
P = 128
NCORES = 8
TP = 4  # cores per batch group
HPC = H // TP  # heads per core = 4
SL = S // TP  # tokens owned per core = 512 (4 strided blocks of 128)
QT = 512  # query row tile (attention row granularity)
SUB = 256  # score/exp subtile width (parity pair = 1 PSUM bank)
KB = 128  # key block
NCK = 4  # reduce-scatter chunks
NEG = -1.0e9  # additive causal mask (exp underflows to exactly 0)
NQ = 256  # FFN2 output-column tile (1 PSUM bank per accumulator)

f32 = mybir.dt.float32
f32r = mybir.dt.float32r
bf16 = mybir.dt.bfloat16
bfnp = ml_dtypes.bfloat16

REPLICA_GROUPS = [[0, 1, 2, 3], [4, 5, 6, 7]]


def _bcast_row_ap(t, row, width):
    """DMA-source AP broadcasting row `row` of DRAM tensor t to 128 partitions."""
    return bass.AP(tensor=t, offset=row * width, ap=[[0, P], [1, width]])


def build_bass():
    import os

    # debug bisection: 1=QKV, 2=+attention rows, 3=+proj/RS, 4=full
    STAGE = int(os.environ.get("KSTAGE", "4"))
    nc = bacc.Bacc("TRN2", target_bir_lowering=False, debug=False, num_devices=NCORES)

    xT = nc.dram_tensor("xT", [D, S], bf16, kind="ExternalInput").ap()
    xs = nc.dram_tensor("xs", [SL, D], f32, kind="ExternalInput").ap()
    wq2 = nc.dram_tensor("wq2", [D, HPC * HS], bf16, kind="ExternalInput").ap()
    wk2 = nc.dram_tensor("wk2", [D, HPC * HS], bf16, kind="ExternalInput").ap()
    wv4 = nc.dram_tensor("wv4", [D, HPC * HS], bf16, kind="ExternalInput").ap()
    wp = nc.dram_tensor("wp", [HPC * HS, D], bf16, kind="ExternalInput").ap()
    w1 = nc.dram_tensor("w1", [D, F], bf16, kind="ExternalInput").ap()
    w2 = nc.dram_tensor("w2", [F, D], bf16, kind="ExternalInput").ap()
    cvec = nc.dram_tensor("cvec", [6, D], f32, kind="ExternalInput").ap()
    b1d = nc.dram_tensor("b1d", [F], f32, kind="ExternalInput").ap()
    out = nc.dram_tensor("out", [SL, D], f32, kind="ExternalOutput").ap()

    # per-chunk collective bounce buffers (separate tensors -> precise deps)
    rs_in = [nc.dram_tensor(f"rs_in{c}", [QT, D], bf16) for c in range(NCK)]
    rs_out = [nc.dram_tensor(f"rs_out{c}", [P, D], bf16) for c in range(NCK)]

    # additive causal mask [all-NEG block | lower-triangle-NEG block]:
    # mfull[:, KB:] is the triangle alone (keep 0 where key t <= query q).
    tri = np.where(
        np.arange(KB)[:, None] <= np.arange(KB)[None, :], 0.0, NEG
    ).astype(np.float32)
    full = np.concatenate([np.full((KB, KB), NEG, np.float32), tri], axis=1)
    m_full_dram = nc.inline_tensor(np.ascontiguousarray(full), name="mask_full")

    with tile.TileContext(nc) as tc:
        with tc.tile_pool(name="const", bufs=1) as constp:
            identb = constp.tile([P, P], bf16)
            make_identity(nc, identb)
            eps_t = constp.tile([P, 1], f32)
            nc.vector.memset(eps_t, EPS)
            b1_sb = constp.tile([P, F // P], f32)
            nc.sync.dma_start(b1_sb, b1d.rearrange("(ko p) -> p ko", p=P))
            g1b = constp.tile([P, D], f32)
            nc.gpsimd.dma_start(g1b, _bcast_row_ap(cvec.tensor, 1, D))
            be1b = constp.tile([P, D], f32)
            nc.gpsimd.dma_start(be1b, _bcast_row_ap(cvec.tensor, 2, D))
            g2b = constp.tile([P, D], f32)
            nc.gpsimd.dma_start(g2b, _bcast_row_ap(cvec.tensor, 3, D))
            be2b = constp.tile([P, D], f32)
            nc.gpsimd.dma_start(be2b, _bcast_row_ap(cvec.tensor, 4, D))
            b2f = constp.tile([P, D], f32)
            nc.gpsimd.dma_start(b2f, _bcast_row_ap(cvec.tensor, 5, D))
            b2b = constp.tile([P, D], bf16)
            nc.vector.tensor_copy(b2b, b2f)

            keep_cm = tc.tile_pool(name="keep", bufs=1)
            keep = keep_cm.__enter__()
            mfull_sb = keep.tile([P, 2 * KB], f32, tag="mfull")
            nc.sync.dma_start(mfull_sb, m_full_dram.ap())
            mtri_sb = mfull_sb[:, KB : 2 * KB]

            wp_sb = keep.tile([P, (HPC * HS) // P, D], bf16, tag="wp")
            nc.sync.dma_start(wp_sb, wp.rearrange("(ko p) n -> p ko n", p=P))

            # attention working set. q/k live on partitions 0-63 with the
            # head-pair parity in the free dim: bf16 matmul operands fault on
            # HW at partition offset 64 (and f32r tile_position row-packing
            # faults when interleaved with bf16 matmuls), so every score
            # matmul reads partition-offset-0 slices.
            q2T = keep.tile([HS, 2, 2, S], bf16, tag="q2T")
            k2T = keep.tile([HS, 2, 2, S], bf16, tag="k2T")
            v4e = keep.tile([P, S // P, HPC * (HS + 1)], bf16, tag="v4e")
            ones4 = keep.tile([P, HPC, 1], bf16, tag="ones4")
            nc.vector.memset(ones4, 1.0)

            # phase B persistents
            w1_sb = keep.tile([P, D // P, F], bf16, tag="w1")
            x1T = keep.tile([P, D // P, SL], bf16, tag="x1T")
            x1r = keep.tile([P, SL // P, D], bf16, tag="x1r")
            hT = keep.tile([P, F // P, SL // 2], bf16, tag="hT")

            # ---------------- Phase QKV ----------------
            wqkv_cm = tc.tile_pool(name="wqkv", bufs=1)
            wqkvp = wqkv_cm.__enter__()
            wq_sb = wqkvp.tile([P, D // P, HPC * HS], bf16, tag="wq")
            nc.sync.dma_start(wq_sb, wq2.rearrange("(ko p) m -> p ko m", p=P))
            wk_sb = wqkvp.tile([P, D // P, HPC * HS], bf16, tag="wk")
            nc.sync.dma_start(wk_sb, wk2.rearrange("(ko p) m -> p ko m", p=P))
            wv_sb = wqkvp.tile([P, D // P, HPC * HS], bf16, tag="wv")
            nc.sync.dma_start(wv_sb, wv4.rearrange("(ko p) m -> p ko m", p=P))

            xr_cm = tc.tile_pool(name="xrp", bufs=2)
            xrp = xr_cm.__enter__()
            with tc.tile_pool(name="ps_qkv", bufs=4, space="PSUM") as psq:
                for tt in range(S // QT):
                    xr = xrp.tile([P, D // P, QT], bf16, tag="xr")
                    nc.sync.dma_start(
                        xr,
                        xT[:, tt * QT : (tt + 1) * QT].rearrange(
                            "(ko p) m -> p ko m", p=P
                        ),
                    )
                    # interleave the big w1 load in 2MB chunks so it never
                    # starves the next xr tile in the DMA rings
                    nc.scalar.dma_start(
                        w1_sb[:, :, tt * (F // 4) : (tt + 1) * (F // 4)],
                        w1[:, tt * (F // 4) : (tt + 1) * (F // 4)].rearrange(
                            "(ko p) m -> p ko m", p=P
                        ),
                    )
                    for hp in range(2):
                        qps = psq.tile([P, QT], f32, tag="qk")
                        for ko in range(D // P):
                            nc.tensor.matmul(
                                qps,
                                wq_sb[:, ko, hp * P : (hp + 1) * P],
                                xr[:, ko, :],
                                start=(ko == 0),
                                stop=(ko == D // P - 1),
                            )
                        for par in range(2):
                            nc.vector.tensor_copy(
                                q2T[:, par, hp, tt * QT : (tt + 1) * QT],
                                qps[par * HS : (par + 1) * HS, :],
                            )
                        kps = psq.tile([P, QT], f32, tag="qk")
                        for ko in range(D // P):
                            nc.tensor.matmul(
                                kps,
                                wk_sb[:, ko, hp * P : (hp + 1) * P],
                                xr[:, ko, :],
                                start=(ko == 0),
                                stop=(ko == D // P - 1),
                            )
                        for par in range(2):
                            nc.vector.tensor_copy(
                                k2T[:, par, hp, tt * QT : (tt + 1) * QT],
                                kps[par * HS : (par + 1) * HS, :],
                            )
                    for mt in range(QT // P):
                        vps = psq.tile([P, HPC * HS], f32, tag="v")
                        for ko in range(D // P):
                            nc.tensor.matmul(
                                vps,
                                xr[:, ko, mt * P : (mt + 1) * P],
                                wv_sb[:, ko, :],
                                start=(ko == 0),
                                stop=(ko == D // P - 1),
                            )
                        idx = tt * (QT // P) + mt
                        vv = v4e[:, idx, :].rearrange("p (h e) -> p h e", e=HS + 1)
                        nc.vector.tensor_copy(
                            vv[:, :, 0:HS],
                            vps.rearrange("p (h e) -> p h e", e=HS),
                        )
                        nc.vector.tensor_copy(vv[:, :, HS : HS + 1], ones4)
            xr_cm.__exit__(None, None, None)
            wqkv_cm.__exit__(None, None, None)

            # ------------- Phase A attention + pipelined phase B -------------
            with (
                tc.tile_pool(name="atp", bufs=1) as atp,
                tc.tile_pool(name="smallp", bufs=2) as smallp,
                tc.tile_pool(name="normp", bufs=1) as normp,
                tc.tile_pool(name="projp", bufs=1) as projp,
                tc.tile_pool(name="ln1p", bufs=1) as ln1p,
                tc.tile_pool(name="w2p", bufs=2) as w2p,
                tc.tile_pool(name="zp", bufs=2) as zp,
                tc.tile_pool(name="ps_sc", bufs=2, space="PSUM") as pssc,
                tc.tile_pool(name="ps_at", bufs=2, space="PSUM") as psat,
                tc.tile_pool(name="ps_pr", bufs=1, space="PSUM") as pspr,
                tc.tile_pool(name="ps_b", bufs=2, space="PSUM") as psb,
                tc.tile_pool(name="ps_y", bufs=1, space="PSUM") as psy,
            ):

                def ln1_tile(st):
                    """rs_out[st] + xs[st] (b_proj pre-folded) -> LN1 ->
                    x1r (bf16) and x1T (bf16, transposed)."""
                    yb = ln1p.tile([P, D], bf16, tag="yb")
                    nc.sync.dma_start(yb, rs_out[st].ap())
                    y = ln1p.tile([P, D], f32, tag="y")
                    nc.vector.tensor_copy(y, yb)
                    xst = ln1p.tile([P, D], f32, tag="tmp", name=f"xst_{st}")
                    nc.sync.dma_start(xst, xs[st * P : (st + 1) * P, :])
                    nc.vector.tensor_add(y, y, xst)
                    stats = ln1p.tile([P, 2, 6], f32, tag="stats")
                    yv = y.rearrange("p (s d) -> p s d", s=2)
                    nc.vector.bn_stats(out=stats[:, 0, :], in_=yv[:, 0, :])
                    nc.vector.bn_stats(out=stats[:, 1, :], in_=yv[:, 1, :])
                    mv = ln1p.tile([P, 2], f32, tag="mv")
                    nc.vector.bn_aggr(out=mv, in_=stats)
                    rstd = ln1p.tile([P, 1], f32, tag="rstd")
                    nc.scalar.activation(
                        out=rstd,
                        in_=mv[:, 1:2],
                        func=mybir.ActivationFunctionType.Sqrt,
                        bias=eps_t,
                        scale=1.0,
                    )
                    nc.vector.reciprocal(rstd, rstd)
                    tmp = ln1p.tile([P, D], f32, tag="tmp")
                    nc.vector.tensor_scalar(
                        out=tmp,
                        in0=y,
                        scalar1=mv[:, 0:1],
                        scalar2=rstd,
                        op0=mybir.AluOpType.subtract,
                        op1=mybir.AluOpType.mult,
                    )
                    nc.vector.tensor_mul(tmp, tmp, g1b)
                    nc.vector.tensor_add(x1r[:, st, :], tmp, be1b)
                    for dk in range(D // P):
                        tp = psb.tile([P, P], bf16, tag="scr", name=f"tp_{st}_{dk}")
                        nc.tensor.transpose(
                            tp, x1r[:, st, dk * P : (dk + 1) * P], identb
                        )
                        nc.vector.tensor_copy(x1T[:, dk, st * P : (st + 1) * P], tp)

                def ffn1_half(h):
                    """hT = relu(w1.T @ x1T[:, :, half h] + b1)."""
                    tsl = slice(h * (SL // 2), (h + 1) * (SL // 2))
                    for ft in range(F // P):
                        hps = psb.tile(
                            [P, SL // 2],
                            f32,
                            tag="scr",
                            name=f"hps_{h}_{ft}",
                            padded_shape=[P, QT],
                        )
                        for ko in range(D // P):
                            nc.tensor.matmul(
                                hps,
                                w1_sb[:, ko, ft * P : (ft + 1) * P],
                                x1T[:, ko, tsl],
                                start=(ko == 0),
                                stop=(ko == D // P - 1),
                            )
                        nc.scalar.activation(
                            out=hT[:, ft, :],
                            in_=hps,
                            func=mybir.ActivationFunctionType.Relu,
                            bias=b1_sb[:, ft : ft + 1],
                            scale=1.0,
                        )

                def ffn2_half(h):
                    """z = hT.T @ w2 + x1 + b2 -> LN2 -> out, token tiles
                    2h and 2h+1."""
                    zs = []
                    for i in range(2):
                        z = zp.tile([P, D], bf16, tag="z", name=f"z_{h}_{i}")
                        zs.append(z)
                    for dtq in range(D // NQ):
                        w2ts = []
                        for kq in range(2):
                            w2t = w2p.tile(
                                [P, NKO, NQ], bf16, tag="w2t", name=f"w2t_{h}_{dtq}_{kq}"
                            )
                            nc.sync.dma_start(
                                w2t,
                                w2[
                                    kq * (F // 2) : (kq + 1) * (F // 2),
                                    dtq * NQ : (dtq + 1) * NQ,
                                ].rearrange("(ko p) n -> p ko n", p=P),
                            )
                            w2ts.append(w2t)
                        dsl = slice(dtq * NQ, (dtq + 1) * NQ)
                        for mtl in range(2):
                            ypss = psy.tile([P, NQ], f32, tag="yq")
                            for kq in range(2):
                                for ko in range(NKO):
                                    nc.tensor.matmul(
                                        ypss,
                                        hT[:, kq * NKO + ko, mtl * P : (mtl + 1) * P],
                                        w2ts[kq][:, ko, :],
                                        start=(kq == 0 and ko == 0),
                                        stop=(kq == 1 and ko == NKO - 1),
                                    )
                            nc.vector.tensor_copy(zs[mtl][:, dsl], ypss)
                            nc.vector.tensor_add(
                                zs[mtl][:, dsl], zs[mtl][:, dsl], x1r[:, 2 * h + mtl, dsl]
                            )
                    for mtl in range(2):
                        st = 2 * h + mtl
                        zm = zs[mtl]
                        nc.vector.tensor_add(zm, zm, b2b)
                        stats = ln1p.tile([P, 2, 6], f32, tag="stats")
                        zv = zm.rearrange("p (s d) -> p s d", s=2)
                        nc.vector.bn_stats(out=stats[:, 0, :], in_=zv[:, 0, :])
                        nc.vector.bn_stats(out=stats[:, 1, :], in_=zv[:, 1, :])
                        mv = ln1p.tile([P, 2], f32, tag="mv")
                        nc.vector.bn_aggr(out=mv, in_=stats)
                        rstd = ln1p.tile([P, 1], f32, tag="rstd")
                        nc.scalar.activation(
                            out=rstd,
                            in_=mv[:, 1:2],
                            func=mybir.ActivationFunctionType.Sqrt,
                            bias=eps_t,
                            scale=1.0,
                        )
                        nc.vector.reciprocal(rstd, rstd)
                        o = ln1p.tile([P, D], f32, tag="tmp", name=f"o_{h}_{mtl}")
                        nc.vector.tensor_scalar(
                            out=o,
                            in0=zm,
                            scalar1=mv[:, 0:1],
                            scalar2=rstd,
                            op0=mybir.AluOpType.subtract,
                            op1=mybir.AluOpType.mult,
                        )
                        nc.vector.tensor_mul(o, o, g2b)
                        nc.vector.tensor_add(o, o, be2b)
                        nc.sync.dma_start(out[st * P : (st + 1) * P, :], o)

                NKO = F // (2 * P)  # 16 k-subtiles per streamed w2 tile

                for qt in range(S // QT if STAGE >= 2 else 0):
                    nkb = 4 * qt + 4
                    qsl = slice(qt * QT, (qt + 1) * QT)
                    attnT = atp.tile([P, 2, QT], bf16, tag="attnT", name=f"attnT_{qt}")
                    for hp in range(2):
                        apair = psat.tile([HS + 1, QT], f32, tag="at")
                        apodd = psat.tile([HS + 1, QT], f32, tag="at")
                        for sub in range(QT // SUB):
                            qlo = qt * QT + sub * SUB
                            live = [kb for kb in range(nkb) if KB * kb < qlo + SUB]
                            ssl = slice(sub * SUB, (sub + 1) * SUB)
                            for j, kb in enumerate(live):
                                ksl = slice(kb * KB, (kb + 1) * KB)
                                sp = pssc.tile([P, 2, SUB], f32, tag="sc")
                                nc.tensor.matmul(
                                    sp[:, 0, :],
                                    k2T[:, 0, hp, ksl],
                                    q2T[:, 0, hp, qlo : qlo + SUB],
                                    start=True,
                                    stop=True,
                                )
                                nc.tensor.matmul(
                                    sp[:, 1, :],
                                    k2T[:, 1, hp, ksl],
                                    q2T[:, 1, hp, qlo : qlo + SUB],
                                    start=True,
                                    stop=True,
                                )
                                moff = KB * kb - qlo
                                if moff == 0:
                                    nc.vector.tensor_add(
                                        sp[:, :, 0:KB],
                                        sp[:, :, 0:KB],
                                        mtri_sb[:, None, :].to_broadcast((P, 2, KB)),
                                    )
                                elif moff == KB:
                                    nc.vector.tensor_add(
                                        sp,
                                        sp,
                                        mfull_sb[:, None, :].to_broadcast(
                                            (P, 2, SUB)
                                        ),
                                    )
                                ee = smallp.tile([P, 2, SUB], bf16, tag="ee")
                                nc.scalar.activation(
                                    out=ee,
                                    in_=sp,
                                    func=mybir.ActivationFunctionType.Exp,
                                    scale=float(HS) ** -0.5,
                                )
                                he = (2 * hp) * (HS + 1)
                                ho = (2 * hp + 1) * (HS + 1)
                                nc.tensor.matmul(
                                    apair[:, ssl],
                                    v4e[:, kb, he : he + HS + 1],
                                    ee[:, 0, :],
                                    start=(j == 0),
                                    stop=(j == len(live) - 1),
                                )
                                nc.tensor.matmul(
                                    apodd[:, ssl],
                                    v4e[:, kb, ho : ho + HS + 1],
                                    ee[:, 1, :],
                                    start=(j == 0),
                                    stop=(j == len(live) - 1),
                                )
                        # normalize in SBUF off the PE critical path
                        for par, aps in ((0, apair), (1, apodd)):
                            ua = normp.tile([HS + 1, QT], f32, tag="ua")
                            nc.vector.tensor_copy(ua, aps)
                            rec = normp.tile([1, QT], f32, tag="rec", bufs=1)
                            nc.vector.reciprocal(rec, ua[HS : HS + 1, :])
                            bc = normp.tile([HS, QT], f32, tag="bc", bufs=1)
                            nc.gpsimd.partition_broadcast(bc, rec)
                            nc.vector.tensor_mul(
                                attnT[par * HS : (par + 1) * HS, hp, :],
                                ua[0:HS, :],
                                bc,
                            )
                    # proj for this qt's 4 token tiles, then RS chunk qt
                    if STAGE < 3:
                        continue
                    for mtl in range(4):
                        mt = 4 * qt + mtl
                        prj = projp.tile([P, D], bf16, tag="prj")
                        for nh in range(D // QT):
                            pps = pspr.tile([P, QT], f32, tag="pr")
                            for ko in range(2):
                                nc.tensor.matmul(
                                    pps,
                                    attnT[:, ko, mtl * P : (mtl + 1) * P],
                                    wp_sb[:, ko, nh * QT : (nh + 1) * QT],
                                    start=(ko == 0),
                                    stop=(ko == 1),
                                )
                            nc.vector.tensor_copy(prj[:, nh * QT : (nh + 1) * QT], pps)
                        nc.sync.dma_start(
                            rs_in[qt].ap()[mtl * P : (mtl + 1) * P, :], prj
                        )
                    nc.gpsimd.collective_compute(
                        "ReduceScatter",
                        mybir.AluOpType.add,
                        replica_groups=REPLICA_GROUPS,
                        ins=[rs_in[qt].ap().opt()],
                        outs=[rs_out[qt].ap().opt()],
                    )
                    # pipelined phase B behind the collective chunks
                    if STAGE >= 4:
                        if qt == 2:
                            ln1_tile(0)
                            ln1_tile(1)
                        elif qt == 3:
                            ffn1_half(0)
                            ffn2_half(0)

                if STAGE >= 4:
                    # tail: remaining token tiles
                    ln1_tile(2)
                    ln1_tile(3)
                    ffn1_half(1)
                    ffn2_half(1)
                else:
                    dummy = ln1p.tile([P, D], f32, tag="tmp", name="dummy")
                    nc.vector.memset(dummy, 0.0)
                    for st in range(SL // P):
                        nc.sync.dma_start(out[st * P : (st + 1) * P, :], dummy)

            keep_cm.__exit__(None, None, None)

    nc.compile()
    return nc


_NC_CACHE = []


def _get_nc():
    if not _NC_CACHE:
        _NC_CACHE.append(build_bass())
    return _NC_CACHE[0]


def _token_blocks(r):
    """Global token rows (within a batch element) owned by rank r, as NCK
    blocks of 128: block ck covers rows [512*ck + 128*r, 512*ck + 128*r + 128)."""
    return [slice(QT * ck + P * r, QT * ck + P * r + P) for ck in range(NCK)]


def make_in_maps(x, wq, wk, wv, w_proj, b_proj, w1, b1, w2, b2, g1, be1, g2, be2):
    x = np.asarray(x, dtype=np.float32)
    bp = np.asarray(b_proj, dtype=np.float32)
    cat = lambda w, h0: np.ascontiguousarray(
        np.concatenate(
            [np.asarray(w[h0 + i], dtype=np.float32) for i in range(HPC)], axis=1
        )
    ).astype(bfnp)
    cvec_rows = [b_proj, g1, be1, g2, be2, b2]
    cvec = np.ascontiguousarray(
        np.stack([np.asarray(v, dtype=np.float32) for v in cvec_rows])
    )
    w1c = np.ascontiguousarray(np.asarray(w1, dtype=np.float32)).astype(bfnp)
    w2c = np.ascontiguousarray(np.asarray(w2, dtype=np.float32)).astype(bfnp)
    b1c = np.ascontiguousarray(np.asarray(b1, dtype=np.float32))
    wpc = np.asarray(w_proj, dtype=np.float32)
    xTs = [np.ascontiguousarray(x[g].T.astype(bfnp)) for g in range(B)]
    in_maps = []
    for c in range(NCORES):
        g, r = divmod(c, TP)
        h0 = HPC * r
        # b_proj is folded into the residual here instead of on device
        xs_blocks = np.concatenate(
            [x[g, blk] + bp for blk in _token_blocks(r)], axis=0
        )
        in_maps.append(
            {
                "xT": xTs[g],
                "xs": np.ascontiguousarray(xs_blocks),
                "wq2": cat(wq, h0),
                "wk2": cat(wk, h0),
                "wv4": cat(wv, h0),
                "wp": np.ascontiguousarray(
                    wpc[HPC * HS * r : HPC * HS * (r + 1)].astype(bfnp)
                ),
                "w1": w1c,
                "w2": w2c,
                "cvec": cvec,
                "b1d": b1c,
            }
        )
    return in_maps


def assemble(results):
    full = np.empty((B, S, D), dtype=np.float32)
    for c in range(NCORES):
        g, r = divmod(c, TP)
        o = results[c]["out"]
        for ck, blk in enumerate(_token_blocks(r)):
            full[g, blk] = o[ck * P : (ck + 1) * P]
    return full


def kernel(**inputs):
    nc = _get_nc()
    in_maps = make_in_maps(**inputs)
    res = run_bass_kernel_spmd(nc, in_maps, core_ids=list(range(NCORES)))
    return assemble(res.results)


# revision 23
# speedup vs baseline: 1.0706x; 1.0294x over previous
"""Trainium2 Bass kernel for nn_Block_77318001263203 (dense transformer block).

Distribution over 8 NeuronCores: data-parallel over batch (2 groups of 4
cores) x tensor-parallel over heads (4 heads/core) for attention+proj,
4-way-chunked bf16 ReduceScatter of the proj partials over each 4-core
group (each chunk hands every rank one 128-token block, so rank r owns
the strided token set {512*ck + 128*r + j}), then token-parallel FFN with
replicated FFN weights — no second collective.

v2 vs the original: all matmuls run in bf16 (weights cast host-side,
activations cast on device; fp32 PSUM accumulate), halving HBM/collective
bytes and PE power. The attention score matmuls drop the float32r
tile_position packing (that packing faults on hardware with bf16) and run
as plain K=64 matmuls. Phase B (LN1/transpose/FFN/LN2) is pipelined into
the attention tail by emission order: LN1 of token tiles 0-1 is emitted
behind ReduceScatter chunk 2, FFN half 0 behind chunk 3, keeping the PE
busy through the collectives and the HAM activity window warm. b_proj is
pre-added into the xs residual host-side.

kernel(**inputs) takes the FULL inputs from setup_inputs() and returns the
FULL [2, 2048, 1024] float32 output.
"""

import numpy as np
import ml_dtypes

import concourse.bass as bass
import concourse.mybir as mybir
import concourse.tile as tile
from concourse import bacc
from concourse.bass_utils import run_bass_kernel_spmd
from concourse.masks import make_identity

# problem dims (hardcoded per the harness contract)
B, S, D = 2, 2048, 1024
H, HS, F = 16, 64, 4096
EPS = 1e-5
P = 128
NCORES = 8
TP = 4  # cores per batch group
HPC = H // TP  # heads per core = 4
SL = S // TP  # tokens owned per core = 512 (4 strided blocks of 128)
QT = 512  # query row tile (attention row granularity)
SUB = 256  # score/exp subtile width (parity pair = 1 PSUM bank)
KB = 128  # key block
NCK = 4  # reduce-scatter chunks
NEG = -1.0e9  # additive causal mask (exp underflows to exactly 0)
NQ = 256  # FFN2 output-column tile (1 PSUM bank per accumulator)

f32 = mybir.dt.float32
f32r = mybir.dt.float32r
bf16 = mybir.dt.bfloat16
bfnp = ml_dtypes.bfloat16

REPLICA_GROUPS = [[0, 1, 2, 3], [4, 5, 6, 7]]


def _bcast_row_ap(t, row, width):
    """DMA-source AP broadcasting row `row` of DRAM tensor t to 128 partitions."""
    return bass.AP(tensor=t, offset=row * width, ap=[[0, P], [1, width]])


def build_bass():
    import os

    # debug bisection: 1=QKV, 2=+attention rows, 3=+proj/RS, 4=full
    STAGE = int(os.environ.get("KSTAGE", "4"))
    nc = bacc.Bacc("TRN2", target_bir_lowering=False, debug=False, num_devices=NCORES)

    xT = nc.dram_tensor("xT", [D, S], bf16, kind="ExternalInput").ap()
    xs = nc.dram_tensor("xs", [SL, D], f32, kind="ExternalInput").ap()
    wq2 = nc.dram_tensor("wq2", [D, HPC * HS], bf16, kind="ExternalInput").ap()
    wk2 = nc.dram_tensor("wk2", [D, HPC * HS], bf16, kind="ExternalInput").ap()
    wv4 = nc.dram_tensor("wv4", [D, HPC * HS], bf16, kind="ExternalInput").ap()
    wp = nc.dram_tensor("wp", [HPC * HS, D], bf16, kind="ExternalInput").ap()
    w1 = nc.dram_tensor("w1", [D, F], bf16, kind="ExternalInput").ap()
    w2 = nc.dram_tensor("w2", [F, D], bf16, kind="ExternalInput").ap()
    cvec = nc.dram_tensor("cvec", [6, D], f32, kind="ExternalInput").ap()
    b1d = nc.dram_tensor("b1d", [F], f32, kind="ExternalInput").ap()
    out = nc.dram_tensor("out", [SL, D], f32, kind="ExternalOutput").ap()

    # per-chunk collective bounce buffers (separate tensors -> precise deps)
    rs_in = [nc.dram_tensor(f"rs_in{c}", [QT, D], bf16) for c in range(NCK)]
    rs_out = [nc.dram_tensor(f"rs_out{c}", [P, D], bf16) for c in range(NCK)]

    # additive causal mask [all-NEG block | lower-triangle-NEG block]:
    # mfull[:, KB:] is the triangle alone (keep 0 where key t <= query q).
    tri = np.where(
        np.arange(KB)[:, None] <= np.arange(KB)[None, :], 0.0, NEG
    ).astype(np.float32)
    full = np.concatenate([np.full((KB, KB), NEG, np.float32), tri], axis=1)
    m_full_dram = nc.inline_tensor(np.ascontiguousarray(full), name="mask_full")

    with tile.TileContext(nc) as tc:
        with tc.tile_pool(name="const", bufs=1) as constp:
            identb = constp.tile([P, P], bf16)
            make_identity(nc, identb)
            eps_t = constp.tile([P, 1], f32)
            nc.vector.memset(eps_t, EPS)
            b1_sb = constp.tile([P, F // P], f32)
            nc.sync.dma_start(b1_sb, b1d.rearrange("(ko p) -> p ko", p=P))
            g1b = constp.tile([P, D], f32)
            nc.gpsimd.dma_start(g1b, _bcast_row_ap(cvec.tensor, 1, D))
            be1b = constp.tile([P, D], f32)
            nc.gpsimd.dma_start(be1b, _bcast_row_ap(cvec.tensor, 2, D))
            g2b = constp.tile([P, D], f32)
            nc.gpsimd.dma_start(g2b, _bcast_row_ap(cvec.tensor, 3, D))
            be2b = constp.tile([P, D], f32)
            nc.gpsimd.dma_start(be2b, _bcast_row_ap(cvec.tensor, 4, D))
            b2f = constp.tile([P, D], f32)
            nc.gpsimd.dma_start(b2f, _bcast_row_ap(cvec.tensor, 5, D))
            b2b = constp.tile([P, D], bf16)
            nc.vector.tensor_copy(b2b, b2f)

            keep_cm = tc.tile_pool(name="keep", bufs=1)
            keep = keep_cm.__enter__()
            mfull_sb = keep.tile([P, 2 * KB], f32, tag="mfull")
            nc.sync.dma_start(mfull_sb, m_full_dram.ap())
            mtri_sb = mfull_sb[:, KB : 2 * KB]

            wp_sb = keep.tile([P, (HPC * HS) // P, D], bf16, tag="wp")
            nc.sync.dma_start(wp_sb, wp.rearrange("(ko p) n -> p ko n", p=P))

            # attention working set. q/k live on partitions 0-63 with the
            # head-pair parity in the free dim: bf16 matmul operands fault on
            # HW at partition offset 64 (and f32r tile_position row-packing
            # faults when interleaved with bf16 matmuls), so every score
            # matmul reads partition-offset-0 slices.
            q2T = keep.tile([HS, 2, 2, S], bf16, tag="q2T")
            k2T = keep.tile([HS, 2, 2, S], bf16, tag="k2T")
            v4e = keep.tile([P, S // P, HPC * (HS + 1)], bf16, tag="v4e")
            ones4 = keep.tile([P, HPC, 1], bf16, tag="ones4")
            nc.vector.memset(ones4, 1.0)

            # phase B persistents
            w1_sb = keep.tile([P, D // P, F], bf16, tag="w1")
            x1T = keep.tile([P, D // P, SL], bf16, tag="x1T")
            x1r = keep.tile([P, SL // P, D], bf16, tag="x1r")
            hT = keep.tile([P, F // P, SL // 2], bf16, tag="hT")

            # ---------------- Phase QKV ----------------
            wqkv_cm = tc.tile_pool(name="wqkv", bufs=1)
            wqkvp = wqkv_cm.__enter__()
            wq_sb = wqkvp.tile([P, D // P, HPC * HS], bf16, tag="wq")
            nc.sync.dma_start(wq_sb, wq2.rearrange("(ko p) m -> p ko m", p=P))
            wk_sb = wqkvp.tile([P, D // P, HPC * HS], bf16, tag="wk")
            nc.sync.dma_start(wk_sb, wk2.rearrange("(ko p) m -> p ko m", p=P))
            wv_sb = wqkvp.tile([P, D // P, HPC * HS], bf16, tag="wv")
            nc.sync.dma_start(wv_sb, wv4.rearrange("(ko p) m -> p ko m", p=P))

            xr_cm = tc.tile_pool(name="xrp", bufs=2)
            xrp = xr_cm.__enter__()
            with tc.tile_pool(name="ps_qkv", bufs=4, space="PSUM") as psq:
                for tt in range(S // QT):
                    xr = xrp.tile([P, D // P, QT], bf16, tag="xr")
                    nc.sync.dma_start(
                        xr,
                        xT[:, tt * QT : (tt + 1) * QT].rearrange(
                            "(ko p) m -> p ko m", p=P
                        ),
                    )
                    # interleave the big w1 load in 2MB chunks so it never
                    # starves the next xr tile in the DMA rings
                    nc.scalar.dma_start(
                        w1_sb[:, :, tt * (F // 4) : (tt + 1) * (F // 4)],
                        w1[:, tt * (F // 4) : (tt + 1) * (F // 4)].rearrange(
                            "(ko p) m -> p ko m", p=P
                        ),
                    )
                    for hp in range(2):
                        qps = psq.tile([P, QT], f32, tag="qk")
                        for ko in range(D // P):
                            nc.tensor.matmul(
                                qps,
                                wq_sb[:, ko, hp * P : (hp + 1) * P],
                                xr[:, ko, :],
                                start=(ko == 0),
                                stop=(ko == D // P - 1),
                            )
                        for par in range(2):
                            nc.vector.tensor_copy(
                                q2T[:, par, hp, tt * QT : (tt + 1) * QT],
                                qps[par * HS : (par + 1) * HS, :],
                            )
                        kps = psq.tile([P, QT], f32, tag="qk")
                        for ko in range(D // P):
                            nc.tensor.matmul(
                                kps,
                                wk_sb[:, ko, hp * P : (hp + 1) * P],
                                xr[:, ko, :],
                                start=(ko == 0),
                                stop=(ko == D // P - 1),
                            )
                        for par in range(2):
                            nc.vector.tensor_copy(
                                k2T[:, par, hp, tt * QT : (tt + 1) * QT],
                                kps[par * HS : (par + 1) * HS, :],
                            )
                    for mt in range(QT // P):
                        vps = psq.tile([P, HPC * HS], f32, tag="v")
                        for ko in range(D // P):
                            nc.tensor.matmul(
                                vps,
                                xr[:, ko, mt * P : (mt + 1) * P],
                                wv_sb[:, ko, :],
                                start=(ko == 0),
                                stop=(ko == D // P - 1),
                            )
                        idx = tt * (QT // P) + mt
                        vv = v4e[:, idx, :].rearrange("p (h e) -> p h e", e=HS + 1)
                        nc.vector.tensor_copy(
                            vv[:, :, 0:HS],
                            vps.rearrange("p (h e) -> p h e", e=HS),
                        )
                        nc.vector.tensor_copy(vv[:, :, HS : HS + 1], ones4)
            xr_cm.__exit__(None, None, None)
            wqkv_cm.__exit__(None, None, None)

            # ------------- Phase A attention + pipelined phase B -------------
            with (
                tc.tile_pool(name="atp", bufs=1) as atp,
                tc.tile_pool(name="smallp", bufs=3) as smallp,
                tc.tile_pool(name="normp", bufs=1) as normp,
                tc.tile_pool(name="projp", bufs=1) as projp,
                tc.tile_pool(name="ln1p", bufs=1) as ln1p,
                tc.tile_pool(name="w2p", bufs=2) as w2p,
                tc.tile_pool(name="zp", bufs=2) as zp,
                tc.tile_pool(name="ps_sc", bufs=3, space="PSUM") as pssc,
                tc.tile_pool(name="ps_at", bufs=2, space="PSUM") as psat,
                tc.tile_pool(name="ps_b", bufs=2, space="PSUM") as psb,
                tc.tile_pool(name="ps_y", bufs=1, space="PSUM") as psy,
            ):

                def ln1_tile(st):
                    """rs_out[st] + xs[st] (b_proj pre-folded) -> LN1 ->
                    x1r (bf16) and x1T (bf16, transposed)."""
                    yb = ln1p.tile([P, D], bf16, tag="yb")
                    nc.sync.dma_start(yb, rs_out[st].ap())
                    y = ln1p.tile([P, D], f32, tag="y")
                    nc.vector.tensor_copy(y, yb)
                    xst = ln1p.tile([P, D], f32, tag="tmp", name=f"xst_{st}")
                    nc.sync.dma_start(xst, xs[st * P : (st + 1) * P, :])
                    nc.vector.tensor_add(y, y, xst)
                    stats = ln1p.tile([P, 2, 6], f32, tag="stats")
                    yv = y.rearrange("p (s d) -> p s d", s=2)
                    nc.vector.bn_stats(out=stats[:, 0, :], in_=yv[:, 0, :])
                    nc.vector.bn_stats(out=stats[:, 1, :], in_=yv[:, 1, :])
                    mv = ln1p.tile([P, 2], f32, tag="mv")
                    nc.vector.bn_aggr(out=mv, in_=stats)
                    rstd = ln1p.tile([P, 1], f32, tag="rstd")
                    nc.scalar.activation(
                        out=rstd,
                        in_=mv[:, 1:2],
                        func=mybir.ActivationFunctionType.Sqrt,
                        bias=eps_t,
                        scale=1.0,
                    )
                    nc.vector.reciprocal(rstd, rstd)
                    tmp = ln1p.tile([P, D], f32, tag="tmp")
                    nc.vector.tensor_scalar(
                        out=tmp,
                        in0=y,
                        scalar1=mv[:, 0:1],
                        scalar2=rstd,
                        op0=mybir.AluOpType.subtract,
                        op1=mybir.AluOpType.mult,
                    )
                    nc.vector.tensor_mul(tmp, tmp, g1b)
                    nc.vector.tensor_add(x1r[:, st, :], tmp, be1b)
                    for dk in range(D // P):
                        tp = psb.tile([P, P], bf16, tag="scr", name=f"tp_{st}_{dk}")
                        nc.tensor.transpose(
                            tp, x1r[:, st, dk * P : (dk + 1) * P], identb
                        )
                        nc.vector.tensor_copy(x1T[:, dk, st * P : (st + 1) * P], tp)

                def ffn1_half(h):
                    """hT = relu(w1.T @ x1T[:, :, half h] + b1)."""
                    tsl = slice(h * (SL // 2), (h + 1) * (SL // 2))
                    for ft in range(F // P):
                        hps = psb.tile(
                            [P, SL // 2],
                            f32,
                            tag="scr",
                            name=f"hps_{h}_{ft}",
                            padded_shape=[P, QT],
                        )
                        for ko in range(D // P):
                            nc.tensor.matmul(
                                hps,
                                w1_sb[:, ko, ft * P : (ft + 1) * P],
                                x1T[:, ko, tsl],
                                start=(ko == 0),
                                stop=(ko == D // P - 1),
                            )
                        nc.scalar.activation(
                            out=hT[:, ft, :],
                            in_=hps,
                            func=mybir.ActivationFunctionType.Relu,
                            bias=b1_sb[:, ft : ft + 1],
                            scale=1.0,
                        )

                def ffn2_half(h):
                    """z = hT.T @ w2 + x1 + b2 -> LN2 -> out, token tiles
                    2h and 2h+1."""
                    zs = []
                    for i in range(2):
                        z = zp.tile([P, D], bf16, tag="z", name=f"z_{h}_{i}")
                        zs.append(z)
                    for dtq in range(D // NQ):
                        w2ts = []
                        for kq in range(2):
                            w2t = w2p.tile(
                                [P, NKO, NQ], bf16, tag="w2t", name=f"w2t_{h}_{dtq}_{kq}"
                            )
                            nc.sync.dma_start(
                                w2t,
                                w2[
                                    kq * (F // 2) : (kq + 1) * (F // 2),
                                    dtq * NQ : (dtq + 1) * NQ,
                                ].rearrange("(ko p) n -> p ko n", p=P),
                            )
                            w2ts.append(w2t)
                        dsl = slice(dtq * NQ, (dtq + 1) * NQ)
                        for mtl in range(2):
                            ypss = psy.tile([P, NQ], f32, tag="yq")
                            for kq in range(2):
                                for ko in range(NKO):
                                    nc.tensor.matmul(
                                        ypss,
                                        hT[:, kq * NKO + ko, mtl * P : (mtl + 1) * P],
                                        w2ts[kq][:, ko, :],
                                        start=(kq == 0 and ko == 0),
                                        stop=(kq == 1 and ko == NKO - 1),
                                    )
                            nc.vector.tensor_copy(zs[mtl][:, dsl], ypss)
                            nc.vector.tensor_add(
                                zs[mtl][:, dsl], zs[mtl][:, dsl], x1r[:, 2 * h + mtl, dsl]
                            )
                    for mtl in range(2):
                        st = 2 * h + mtl
                        zm = zs[mtl]
                        nc.vector.tensor_add(zm, zm, b2b)
                        stats = ln1p.tile([P, 2, 6], f32, tag="stats")
                        zv = zm.rearrange("p (s d) -> p s d", s=2)
                        nc.vector.bn_stats(out=stats[:, 0, :], in_=zv[:, 0, :])
                        nc.vector.bn_stats(out=stats[:, 1, :], in_=zv[:, 1, :])
                        mv = ln1p.tile([P, 2], f32, tag="mv")
                        nc.vector.bn_aggr(out=mv, in_=stats)
                        rstd = ln1p.tile([P, 1], f32, tag="rstd")
                        nc.scalar.activation(
                            out=rstd,
                            in_=mv[:, 1:2],
                            func=mybir.ActivationFunctionType.Sqrt,
                            bias=eps_t,
                            scale=1.0,
                        )
                        nc.vector.reciprocal(rstd, rstd)
                        o = ln1p.tile([P, D], f32, tag="tmp", name=f"o_{h}_{mtl}")
                        nc.vector.tensor_scalar(
                            out=o,
                            in0=zm,
                            scalar1=mv[:, 0:1],
                            scalar2=rstd,
                            op0=mybir.AluOpType.subtract,
                            op1=mybir.AluOpType.mult,
                        )
                        nc.vector.tensor_mul(o, o, g2b)
                        nc.vector.tensor_add(o, o, be2b)
                        nc.sync.dma_start(out[st * P : (st + 1) * P, :], o)

                NKO = F // (2 * P)  # 16 k-subtiles per streamed w2 tile

                for qt in range(S // QT if STAGE >= 2 else 0):
                    nkb = 4 * qt + 4
                    qsl = slice(qt * QT, (qt + 1) * QT)
                    attnT = atp.tile([P, 2, QT], bf16, tag="attnT", name=f"attnT_{qt}")
                    for hp in range(2):
                        apair = psat.tile([HS + 1, QT], f32, tag="at")
                        apodd = psat.tile([HS + 1, QT], f32, tag="at")
                        for sub in range(QT // SUB):
                            qlo = qt * QT + sub * SUB
                            live = [kb for kb in range(nkb) if KB * kb < qlo + SUB]
                            ssl = slice(sub * SUB, (sub + 1) * SUB)
                            n = len(live)
                            ees = [None] * n

                            def emit_av(i):
                                kb = live[i]
                                he = (2 * hp) * (HS + 1)
                                ho = (2 * hp + 1) * (HS + 1)
                                nc.tensor.matmul(
                                    apair[:, ssl],
                                    v4e[:, kb, he : he + HS + 1],
                                    ees[i][:, 0, :],
                                    start=(i == 0),
                                    stop=(i == n - 1),
                                )
                                nc.tensor.matmul(
                                    apodd[:, ssl],
                                    v4e[:, kb, ho : ho + HS + 1],
                                    ees[i][:, 1, :],
                                    start=(i == 0),
                                    stop=(i == n - 1),
                                )

                            # software-pipelined: av matmuls trail the
                            # score->exp chain by 2 key blocks so the exp
                            # latency hides behind the next blocks' scores
                            for j, kb in enumerate(live):
                                ksl = slice(kb * KB, (kb + 1) * KB)
                                sp = pssc.tile(
                                    [P, 2, SUB],
                                    f32,
                                    tag="sc",
                                    name=f"sp_{qt}_{hp}_{sub}_{j}",
                                )
                                nc.tensor.matmul(
                                    sp[:, 0, :],
                                    k2T[:, 0, hp, ksl],
                                    q2T[:, 0, hp, qlo : qlo + SUB],
                                    start=True,
                                    stop=True,
                                )
                                nc.tensor.matmul(
                                    sp[:, 1, :],
                                    k2T[:, 1, hp, ksl],
                                    q2T[:, 1, hp, qlo : qlo + SUB],
                                    start=True,
                                    stop=True,
                                )
                                moff = KB * kb - qlo
                                if moff == 0:
                                    nc.vector.tensor_add(
                                        sp[:, :, 0:KB],
                                        sp[:, :, 0:KB],
                                        mtri_sb[:, None, :].to_broadcast((P, 2, KB)),
                                    )
                                elif moff == KB:
                                    nc.vector.tensor_add(
                                        sp,
                                        sp,
                                        mfull_sb[:, None, :].to_broadcast(
                                            (P, 2, SUB)
                                        ),
                                    )
                                ee = smallp.tile(
                                    [P, 2, SUB],
                                    bf16,
                                    tag="ee",
                                    name=f"ee_{qt}_{hp}_{sub}_{j}",
                                )
                                nc.scalar.activation(
                                    out=ee,
                                    in_=sp,
                                    func=mybir.ActivationFunctionType.Exp,
                                    scale=float(HS) ** -0.5,
                                )
                                ees[j] = ee
                                if j >= 2:
                                    emit_av(j - 2)
                            if n >= 2:
                                emit_av(n - 2)
                            emit_av(n - 1)
                        # normalize in SBUF off the PE critical path
                        for par, aps in ((0, apair), (1, apodd)):
                            ua = normp.tile([HS + 1, QT], f32, tag="ua")
                            nc.vector.tensor_copy(ua, aps)
                            rec = normp.tile([1, QT], f32, tag="rec", bufs=1)
                            nc.vector.reciprocal(rec, ua[HS : HS + 1, :])
                            bc = normp.tile([HS, QT], f32, tag="bc", bufs=1)
                            nc.gpsimd.partition_broadcast(bc, rec)
                            nc.vector.tensor_mul(
                                attnT[par * HS : (par + 1) * HS, hp, :],
                                ua[0:HS, :],
                                bc,
                            )
                    # LN1 of token tile 0 rides the last attention row: by
                    # now RS chunk 0 has long completed, so its DVE ops do
                    # not block the queue
                    if STAGE >= 4 and qt == 3:
                        ln1_tile(0)
                    # proj for this qt's 4 token tiles, then RS chunk qt
                    if STAGE < 3:
                        continue
                    for mtl in range(4):
                        mt = 4 * qt + mtl
                        prj = projp.tile([P, D], bf16, tag="prj")
                        for nh in range(D // QT):
                            pps = pssc.tile(
                                [P, QT], f32, tag="sc", name=f"pps_{mt}_{nh}"
                            )
                            for ko in range(2):
                                nc.tensor.matmul(
                                    pps,
                                    attnT[:, ko, mtl * P : (mtl + 1) * P],
                                    wp_sb[:, ko, nh * QT : (nh + 1) * QT],
                                    start=(ko == 0),
                                    stop=(ko == 1),
                                )
                            nc.vector.tensor_copy(prj[:, nh * QT : (nh + 1) * QT], pps)
                        nc.sync.dma_start(
                            rs_in[qt].ap()[mtl * P : (mtl + 1) * P, :], prj
                        )
                    nc.gpsimd.collective_compute(
                        "ReduceScatter",
                        mybir.AluOpType.add,
                        replica_groups=REPLICA_GROUPS,
                        ins=[rs_in[qt].ap().opt()],
                        outs=[rs_out[qt].ap().opt()],
                    )
                    # pipelined phase B behind the last collective chunk
                    if STAGE >= 4 and qt == 3:
                        ln1_tile(1)
                        ffn1_half(0)
                        ffn2_half(0)

                if STAGE >= 4:
                    # tail: remaining token tiles
                    ln1_tile(2)
                    ln1_tile(3)
                    ffn1_half(1)
                    ffn2_half(1)
                else:
                    dummy = ln1p.tile([P, D], f32, tag="tmp", name="dummy")
                    nc.vector.memset(dummy, 0.0)
                    for st in range(SL // P):
                        nc.sync.dma_start(out[st * P : (st + 1) * P, :], dummy)

            keep_cm.__exit__(None, None, None)

    nc.compile()
    return nc


_NC_CACHE = []


def _get_nc():
    if not _NC_CACHE:
        _NC_CACHE.append(build_bass())
    return _NC_CACHE[0]


def _token_blocks(r):
    """Global token rows (within a batch element) owned by rank r, as NCK
    blocks of 128: block ck covers rows [512*ck + 128*r, 512*ck + 128*r + 128)."""
    return [slice(QT * ck + P * r, QT * ck + P * r + P) for ck in range(NCK)]


def make_in_maps(x, wq, wk, wv, w_proj, b_proj, w1, b1, w2, b2, g1, be1, g2, be2):
    x = np.asarray(x, dtype=np.float32)
    bp = np.asarray(b_proj, dtype=np.float32)
    cat = lambda w, h0: np.ascontiguousarray(
        np.concatenate(
            [np.asarray(w[h0 + i], dtype=np.float32) for i in range(HPC)], axis=1
        )
    ).astype(bfnp)
    cvec_rows = [b_proj, g1, be1, g2, be2, b2]
    cvec = np.ascontiguousarray(
        np.stack([np.asarray(v, dtype=np.float32) for v in cvec_rows])
    )
    w1c = np.ascontiguousarray(np.asarray(w1, dtype=np.float32)).astype(bfnp)
    w2c = np.ascontiguousarray(np.asarray(w2, dtype=np.float32)).astype(bfnp)
    b1c = np.ascontiguousarray(np.asarray(b1, dtype=np.float32))
    wpc = np.asarray(w_proj, dtype=np.float32)
    xTs = [np.ascontiguousarray(x[g].T.astype(bfnp)) for g in range(B)]
    in_maps = []
    for c in range(NCORES):
        g, r = divmod(c, TP)
        h0 = HPC * r
        # b_proj is folded into the residual here instead of on device
        xs_blocks = np.concatenate(
            [x[g, blk] + bp for blk in _token_blocks(r)], axis=0
        )
        in_maps.append(
            {
                "xT": xTs[g],
                "xs": np.ascontiguousarray(xs_blocks),
                "wq2": cat(wq, h0),
                "wk2": cat(wk, h0),
                "wv4": cat(wv, h0),
                "wp": np.ascontiguousarray(
                    wpc[HPC * HS * r : HPC * HS * (r + 1)].astype(bfnp)
                ),
                "w1": w1c,
                "w2": w2c,
                "cvec": cvec,
                "b1d": b1c,
            }
        )
    return in_maps


def assemble(results):
    full = np.empty((B, S, D), dtype=np.float32)
    for c in range(NCORES):
        g, r = divmod(c, TP)
        o = results[c]["out"]
        for ck, blk in enumerate(_token_blocks(r)):
            full[g, blk] = o[ck * P : (ck + 1) * P]
    return full


def kernel(**inputs):
    nc = _get_nc()
    in_maps = make_in_maps(**inputs)
    res = run_bass_kernel_spmd(nc, in_maps, core_ids=list(range(NCORES)))
    return assemble(res.results)


# revision 30
# speedup vs baseline: 1.0897x; 1.0178x over previous
"""Trainium2 Bass kernel for nn_Block_77318001263203 (dense transformer block).

Distribution over 8 NeuronCores: data-parallel over batch (2 groups of 4
cores) x tensor-parallel over heads (4 heads/core) for attention+proj,
4-way-chunked bf16 ReduceScatter of the proj partials over each 4-core
group (each chunk hands every rank one 128-token block, so rank r owns
the strided token set {512*ck + 128*r + j}), then token-parallel FFN with
replicated FFN weights — no second collective.

v2 vs the original: all matmuls run in bf16 (weights cast host-side,
activations cast on device; fp32 PSUM accumulate), halving HBM/collective
bytes and PE power. The attention score matmuls drop the float32r
tile_position packing (that packing faults on hardware with bf16) and run
as plain K=64 matmuls. Phase B (LN1/transpose/FFN/LN2) is pipelined into
the attention tail by emission order: LN1 of token tiles 0-1 is emitted
behind ReduceScatter chunk 2, FFN half 0 behind chunk 3, keeping the PE
busy through the collectives and the HAM activity window warm. b_proj is
pre-added into the xs residual host-side.

kernel(**inputs) takes the FULL inputs from setup_inputs() and returns the
FULL [2, 2048, 1024] float32 output.
"""

import numpy as np
import ml_dtypes

import concourse.bass as bass
import concourse.mybir as mybir
import concourse.tile as tile
from concourse import bacc
from concourse.bass_utils import run_bass_kernel_spmd
from concourse.masks import make_identity

# problem dims (hardcoded per the harness contract)
B, S, D = 2, 2048, 1024
H, HS, F = 16, 64, 4096
EPS = 1e-5
P = 128
NCORES = 8
TP = 4  # cores per batch group
HPC = H // TP  # heads per core = 4
SL = S // TP  # tokens owned per core = 512 (4 strided blocks of 128)
QT = 512  # query row tile (attention row granularity)
SUB = 256  # score/exp subtile width (parity pair = 1 PSUM bank)
KB = 128  # key block
NCK = 4  # reduce-scatter chunks
NEG = -1.0e9  # additive causal mask (exp underflows to exactly 0)
NQ = 256  # FFN2 output-column tile (1 PSUM bank per accumulator)

f32 = mybir.dt.float32
f32r = mybir.dt.float32r
bf16 = mybir.dt.bfloat16
bfnp = ml_dtypes.bfloat16

REPLICA_GROUPS = [[0, 1, 2, 3], [4, 5, 6, 7]]


def _bcast_row_ap(t, row, width):
    """DMA-source AP broadcasting row `row` of DRAM tensor t to 128 partitions."""
    return bass.AP(tensor=t, offset=row * width, ap=[[0, P], [1, width]])


def build_bass():
    import os

    # debug bisection: 1=QKV, 2=+attention rows, 3=+proj/RS, 4=full
    STAGE = int(os.environ.get("KSTAGE", "4"))
    nc = bacc.Bacc("TRN2", target_bir_lowering=False, debug=False, num_devices=NCORES)

    xT = nc.dram_tensor("xT", [D, S], bf16, kind="ExternalInput").ap()
    xs = nc.dram_tensor("xs", [SL, D], f32, kind="ExternalInput").ap()
    wq2 = nc.dram_tensor("wq2", [D, HPC * HS], bf16, kind="ExternalInput").ap()
    wk2 = nc.dram_tensor("wk2", [D, HPC * HS], bf16, kind="ExternalInput").ap()
    wv4 = nc.dram_tensor("wv4", [D, HPC * HS], bf16, kind="ExternalInput").ap()
    wp = nc.dram_tensor("wp", [HPC * HS, D], bf16, kind="ExternalInput").ap()
    w1 = nc.dram_tensor("w1", [D, F], bf16, kind="ExternalInput").ap()
    w2 = nc.dram_tensor("w2", [F, D], bf16, kind="ExternalInput").ap()
    cvec = nc.dram_tensor("cvec", [6, D], f32, kind="ExternalInput").ap()
    b1d = nc.dram_tensor("b1d", [F], f32, kind="ExternalInput").ap()
    out = nc.dram_tensor("out", [SL, D], f32, kind="ExternalOutput").ap()

    # per-chunk collective bounce buffers (separate tensors -> precise deps)
    rs_in = [nc.dram_tensor(f"rs_in{c}", [QT, D], bf16) for c in range(NCK)]
    rs_out = [nc.dram_tensor(f"rs_out{c}", [P, D], bf16) for c in range(NCK)]

    # additive causal mask [all-NEG block | lower-triangle-NEG block]:
    # mfull[:, KB:] is the triangle alone (keep 0 where key t <= query q).
    tri = np.where(
        np.arange(KB)[:, None] <= np.arange(KB)[None, :], 0.0, NEG
    ).astype(np.float32)
    full = np.concatenate([np.full((KB, KB), NEG, np.float32), tri], axis=1)
    m_full_dram = nc.inline_tensor(np.ascontiguousarray(full), name="mask_full")

    with tile.TileContext(nc) as tc:
        with tc.tile_pool(name="const", bufs=1) as constp:
            identb = constp.tile([P, P], bf16)
            make_identity(nc, identb)
            eps_t = constp.tile([P, 1], f32)
            nc.vector.memset(eps_t, EPS)
            b1_sb = constp.tile([P, F // P], f32)
            nc.sync.dma_start(b1_sb, b1d.rearrange("(ko p) -> p ko", p=P))
            g1b = constp.tile([P, D], f32)
            nc.gpsimd.dma_start(g1b, _bcast_row_ap(cvec.tensor, 1, D))
            be1b = constp.tile([P, D], f32)
            nc.gpsimd.dma_start(be1b, _bcast_row_ap(cvec.tensor, 2, D))
            g2b = constp.tile([P, D], f32)
            nc.gpsimd.dma_start(g2b, _bcast_row_ap(cvec.tensor, 3, D))
            be2b = constp.tile([P, D], f32)
            nc.gpsimd.dma_start(be2b, _bcast_row_ap(cvec.tensor, 4, D))
            b2f = constp.tile([P, D], f32)
            nc.gpsimd.dma_start(b2f, _bcast_row_ap(cvec.tensor, 5, D))
            b2b = constp.tile([P, D], bf16)
            nc.vector.tensor_copy(b2b, b2f)

            keep_cm = tc.tile_pool(name="keep", bufs=1)
            keep = keep_cm.__enter__()
            mfull_sb = keep.tile([P, 2 * KB], f32, tag="mfull")
            nc.sync.dma_start(mfull_sb, m_full_dram.ap())
            mtri_sb = mfull_sb[:, KB : 2 * KB]

            wp_sb = keep.tile([P, (HPC * HS) // P, D], bf16, tag="wp")
            nc.sync.dma_start(wp_sb, wp.rearrange("(ko p) n -> p ko n", p=P))

            # attention working set. q/k live on partitions 0-63 with the
            # head-pair parity in the free dim: bf16 matmul operands fault on
            # HW at partition offset 64 (and f32r tile_position row-packing
            # faults when interleaved with bf16 matmuls), so every score
            # matmul reads partition-offset-0 slices.
            q2T = keep.tile([HS, 2, 2, S], bf16, tag="q2T")
            k2T = keep.tile([HS, 2, 2, S], bf16, tag="k2T")
            v4e = keep.tile([P, S // P, HPC * (HS + 1)], bf16, tag="v4e")
            ones4 = keep.tile([P, HPC, 1], bf16, tag="ones4")
            nc.vector.memset(ones4, 1.0)

            # phase B persistents
            w1_sb = keep.tile([P, D // P, F], bf16, tag="w1")
            x1T = keep.tile([P, D // P, SL], bf16, tag="x1T")
            x1r = keep.tile([P, SL // P, D], bf16, tag="x1r")
            hT = keep.tile([P, F // P, SL // 2], bf16, tag="hT")

            # ---------------- Phase QKV ----------------
            wqkv_cm = tc.tile_pool(name="wqkv", bufs=1)
            wqkvp = wqkv_cm.__enter__()
            wq_sb = wqkvp.tile([P, D // P, HPC * HS], bf16, tag="wq")
            nc.sync.dma_start(wq_sb, wq2.rearrange("(ko p) m -> p ko m", p=P))
            wk_sb = wqkvp.tile([P, D // P, HPC * HS], bf16, tag="wk")
            nc.sync.dma_start(wk_sb, wk2.rearrange("(ko p) m -> p ko m", p=P))
            wv_sb = wqkvp.tile([P, D // P, HPC * HS], bf16, tag="wv")
            nc.sync.dma_start(wv_sb, wv4.rearrange("(ko p) m -> p ko m", p=P))

            xr_cm = tc.tile_pool(name="xrp", bufs=2)
            xrp = xr_cm.__enter__()
            with tc.tile_pool(name="ps_qkv", bufs=4, space="PSUM") as psq:
                for tt in range(S // QT):
                    xr = xrp.tile([P, D // P, QT], bf16, tag="xr")
                    nc.sync.dma_start(
                        xr,
                        xT[:, tt * QT : (tt + 1) * QT].rearrange(
                            "(ko p) m -> p ko m", p=P
                        ),
                    )
                    # interleave the big w1 load in 2MB chunks so it never
                    # starves the next xr tile in the DMA rings
                    nc.scalar.dma_start(
                        w1_sb[:, :, tt * (F // 4) : (tt + 1) * (F // 4)],
                        w1[:, tt * (F // 4) : (tt + 1) * (F // 4)].rearrange(
                            "(ko p) m -> p ko m", p=P
                        ),
                    )
                    for hp in range(2):
                        qps = psq.tile([P, QT], f32, tag="qk")
                        for ko in range(D // P):
                            nc.tensor.matmul(
                                qps,
                                wq_sb[:, ko, hp * P : (hp + 1) * P],
                                xr[:, ko, :],
                                start=(ko == 0),
                                stop=(ko == D // P - 1),
                            )
                        for par in range(2):
                            nc.vector.tensor_copy(
                                q2T[:, par, hp, tt * QT : (tt + 1) * QT],
                                qps[par * HS : (par + 1) * HS, :],
                            )
                        kps = psq.tile([P, QT], f32, tag="qk")
                        for ko in range(D // P):
                            nc.tensor.matmul(
                                kps,
                                wk_sb[:, ko, hp * P : (hp + 1) * P],
                                xr[:, ko, :],
                                start=(ko == 0),
                                stop=(ko == D // P - 1),
                            )
                        for par in range(2):
                            nc.vector.tensor_copy(
                                k2T[:, par, hp, tt * QT : (tt + 1) * QT],
                                kps[par * HS : (par + 1) * HS, :],
                            )
                    for mt in range(QT // P):
                        vps = psq.tile([P, HPC * HS], f32, tag="v")
                        for ko in range(D // P):
                            nc.tensor.matmul(
                                vps,
                                xr[:, ko, mt * P : (mt + 1) * P],
                                wv_sb[:, ko, :],
                                start=(ko == 0),
                                stop=(ko == D // P - 1),
                            )
                        idx = tt * (QT // P) + mt
                        vv = v4e[:, idx, :].rearrange("p (h e) -> p h e", e=HS + 1)
                        nc.vector.tensor_copy(
                            vv[:, :, 0:HS],
                            vps.rearrange("p (h e) -> p h e", e=HS),
                        )
                        nc.vector.tensor_copy(vv[:, :, HS : HS + 1], ones4)
            xr_cm.__exit__(None, None, None)
            wqkv_cm.__exit__(None, None, None)

            # ------------- Phase A attention + pipelined phase B -------------
            with (
                tc.tile_pool(name="atp", bufs=1) as atp,
                tc.tile_pool(name="smallp", bufs=3) as smallp,
                tc.tile_pool(name="normp", bufs=2) as normp,
                tc.tile_pool(name="projp", bufs=1) as projp,
                tc.tile_pool(name="ln1p", bufs=1) as ln1p,
                tc.tile_pool(name="w2p", bufs=2) as w2p,
                tc.tile_pool(name="zp", bufs=2) as zp,
                tc.tile_pool(name="ps_sc", bufs=3, space="PSUM") as pssc,
                tc.tile_pool(name="ps_at", bufs=2, space="PSUM") as psat,
                tc.tile_pool(name="ps_b", bufs=2, space="PSUM") as psb,
                tc.tile_pool(name="ps_y", bufs=1, space="PSUM") as psy,
            ):

                def ln1_tile(st):
                    """rs_out[st] + xs[st] (b_proj pre-folded) -> LN1 ->
                    x1r (bf16) and x1T (bf16, transposed)."""
                    yb = zp.tile([P, D], bf16, tag="z", name=f"yb_{st}")
                    nc.sync.dma_start(yb, rs_out[st].ap())
                    y = ln1p.tile([P, D], f32, tag="y")
                    nc.vector.tensor_copy(y, yb)
                    xst = ln1p.tile([P, D], f32, tag="tmp", name=f"xst_{st}")
                    nc.sync.dma_start(xst, xs[st * P : (st + 1) * P, :])
                    nc.vector.tensor_add(y, y, xst)
                    stats = ln1p.tile([P, 2, 6], f32, tag="stats")
                    yv = y.rearrange("p (s d) -> p s d", s=2)
                    nc.vector.bn_stats(out=stats[:, 0, :], in_=yv[:, 0, :])
                    nc.vector.bn_stats(out=stats[:, 1, :], in_=yv[:, 1, :])
                    mv = ln1p.tile([P, 2], f32, tag="mv")
                    nc.vector.bn_aggr(out=mv, in_=stats)
                    rstd = ln1p.tile([P, 1], f32, tag="rstd")
                    nc.scalar.activation(
                        out=rstd,
                        in_=mv[:, 1:2],
                        func=mybir.ActivationFunctionType.Sqrt,
                        bias=eps_t,
                        scale=1.0,
                    )
                    nc.vector.reciprocal(rstd, rstd)
                    tmp = ln1p.tile([P, D], f32, tag="tmp")
                    nc.vector.tensor_scalar(
                        out=tmp,
                        in0=y,
                        scalar1=mv[:, 0:1],
                        scalar2=rstd,
                        op0=mybir.AluOpType.subtract,
                        op1=mybir.AluOpType.mult,
                    )
                    nc.vector.tensor_mul(tmp, tmp, g1b)
                    nc.vector.tensor_add(x1r[:, st, :], tmp, be1b)
                    for dk in range(D // P):
                        tp = psb.tile([P, P], bf16, tag="scr", name=f"tp_{st}_{dk}")
                        nc.tensor.transpose(
                            tp, x1r[:, st, dk * P : (dk + 1) * P], identb
                        )
                        nc.vector.tensor_copy(x1T[:, dk, st * P : (st + 1) * P], tp)

                def ffn1_quarter(st):
                    """hT[:, :, cols of st] = relu(w1.T @ x1T[:, :, tile st]
                    + b1). Token-tile-granular so it can start as soon as
                    RS chunk st has landed — PE filler during attention."""
                    tsl = slice(st * P, (st + 1) * P)
                    csl = slice((st % 2) * P, (st % 2) * P + P)
                    for ft in range(F // P):
                        hps = psb.tile(
                            [P, P],
                            f32,
                            tag="scr",
                            name=f"hps_{st}_{ft}",
                            padded_shape=[P, QT],
                        )
                        for ko in range(D // P):
                            nc.tensor.matmul(
                                hps,
                                w1_sb[:, ko, ft * P : (ft + 1) * P],
                                x1T[:, ko, tsl],
                                start=(ko == 0),
                                stop=(ko == D // P - 1),
                            )
                        nc.scalar.activation(
                            out=hT[:, ft, csl],
                            in_=hps,
                            func=mybir.ActivationFunctionType.Relu,
                            bias=b1_sb[:, ft : ft + 1],
                            scale=1.0,
                        )

                def ffn2_half(h):
                    """z = hT.T @ w2 + x1 + b2 -> LN2 -> out, token tiles
                    2h and 2h+1."""
                    zs = []
                    for i in range(2):
                        z = zp.tile([P, D], bf16, tag="z", name=f"z_{h}_{i}")
                        zs.append(z)
                    for dtq in range(D // NQ):
                        w2ts = []
                        for kq in range(2):
                            w2t = w2p.tile(
                                [P, NKO, NQ], bf16, tag="w2t", name=f"w2t_{h}_{dtq}_{kq}"
                            )
                            nc.sync.dma_start(
                                w2t,
                                w2[
                                    kq * (F // 2) : (kq + 1) * (F // 2),
                                    dtq * NQ : (dtq + 1) * NQ,
                                ].rearrange("(ko p) n -> p ko n", p=P),
                            )
                            w2ts.append(w2t)
                        dsl = slice(dtq * NQ, (dtq + 1) * NQ)
                        for mtl in range(2):
                            ypss = psy.tile([P, NQ], f32, tag="yq")
                            for kq in range(2):
                                for ko in range(NKO):
                                    nc.tensor.matmul(
                                        ypss,
                                        hT[:, kq * NKO + ko, mtl * P : (mtl + 1) * P],
                                        w2ts[kq][:, ko, :],
                                        start=(kq == 0 and ko == 0),
                                        stop=(kq == 1 and ko == NKO - 1),
                                    )
                            nc.vector.tensor_copy(zs[mtl][:, dsl], ypss)
                            nc.vector.tensor_add(
                                zs[mtl][:, dsl], zs[mtl][:, dsl], x1r[:, 2 * h + mtl, dsl]
                            )
                    for mtl in range(2):
                        st = 2 * h + mtl
                        zm = zs[mtl]
                        nc.vector.tensor_add(zm, zm, b2b)
                        stats = ln1p.tile([P, 2, 6], f32, tag="stats")
                        zv = zm.rearrange("p (s d) -> p s d", s=2)
                        nc.vector.bn_stats(out=stats[:, 0, :], in_=zv[:, 0, :])
                        nc.vector.bn_stats(out=stats[:, 1, :], in_=zv[:, 1, :])
                        mv = ln1p.tile([P, 2], f32, tag="mv")
                        nc.vector.bn_aggr(out=mv, in_=stats)
                        rstd = ln1p.tile([P, 1], f32, tag="rstd")
                        nc.scalar.activation(
                            out=rstd,
                            in_=mv[:, 1:2],
                            func=mybir.ActivationFunctionType.Sqrt,
                            bias=eps_t,
                            scale=1.0,
                        )
                        nc.vector.reciprocal(rstd, rstd)
                        o = ln1p.tile([P, D], f32, tag="tmp", name=f"o_{h}_{mtl}")
                        nc.vector.tensor_scalar(
                            out=o,
                            in0=zm,
                            scalar1=mv[:, 0:1],
                            scalar2=rstd,
                            op0=mybir.AluOpType.subtract,
                            op1=mybir.AluOpType.mult,
                        )
                        nc.vector.tensor_mul(o, o, g2b)
                        nc.vector.tensor_add(o, o, be2b)
                        nc.sync.dma_start(out[st * P : (st + 1) * P, :], o)

                NKO = F // (2 * P)  # 16 k-subtiles per streamed w2 tile

                for qt in range(S // QT if STAGE >= 2 else 0):
                    nkb = 4 * qt + 4
                    qsl = slice(qt * QT, (qt + 1) * QT)
                    attnT = atp.tile([P, 2, QT], bf16, tag="attnT", name=f"attnT_{qt}")
                    for hp in range(2):
                        apair = psat.tile([HS + 1, QT], f32, tag="at")
                        apodd = psat.tile([HS + 1, QT], f32, tag="at")
                        for sub in range(QT // SUB):
                            qlo = qt * QT + sub * SUB
                            live = [kb for kb in range(nkb) if KB * kb < qlo + SUB]
                            ssl = slice(sub * SUB, (sub + 1) * SUB)
                            n = len(live)
                            ees = [None] * n

                            def emit_av(i):
                                kb = live[i]
                                he = (2 * hp) * (HS + 1)
                                ho = (2 * hp + 1) * (HS + 1)
                                nc.tensor.matmul(
                                    apair[:, ssl],
                                    v4e[:, kb, he : he + HS + 1],
                                    ees[i][:, 0, :],
                                    start=(i == 0),
                                    stop=(i == n - 1),
                                )
                                nc.tensor.matmul(
                                    apodd[:, ssl],
                                    v4e[:, kb, ho : ho + HS + 1],
                                    ees[i][:, 1, :],
                                    start=(i == 0),
                                    stop=(i == n - 1),
                                )

                            # software-pipelined: av matmuls trail the
                            # score->exp chain by 2 key blocks so the exp
                            # latency hides behind the next blocks' scores
                            for j, kb in enumerate(live):
                                ksl = slice(kb * KB, (kb + 1) * KB)
                                sp = pssc.tile(
                                    [P, 2, SUB],
                                    f32,
                                    tag="sc",
                                    name=f"sp_{qt}_{hp}_{sub}_{j}",
                                )
                                nc.tensor.matmul(
                                    sp[:, 0, :],
                                    k2T[:, 0, hp, ksl],
                                    q2T[:, 0, hp, qlo : qlo + SUB],
                                    start=True,
                                    stop=True,
                                )
                                nc.tensor.matmul(
                                    sp[:, 1, :],
                                    k2T[:, 1, hp, ksl],
                                    q2T[:, 1, hp, qlo : qlo + SUB],
                                    start=True,
                                    stop=True,
                                )
                                moff = KB * kb - qlo
                                if moff == 0:
                                    nc.vector.tensor_add(
                                        sp[:, :, 0:KB],
                                        sp[:, :, 0:KB],
                                        mtri_sb[:, None, :].to_broadcast((P, 2, KB)),
                                    )
                                elif moff == KB:
                                    nc.vector.tensor_add(
                                        sp,
                                        sp,
                                        mfull_sb[:, None, :].to_broadcast(
                                            (P, 2, SUB)
                                        ),
                                    )
                                ee = smallp.tile(
                                    [P, 2, SUB],
                                    bf16,
                                    tag="ee",
                                    name=f"ee_{qt}_{hp}_{sub}_{j}",
                                )
                                nc.scalar.activation(
                                    out=ee,
                                    in_=sp,
                                    func=mybir.ActivationFunctionType.Exp,
                                    scale=float(HS) ** -0.5,
                                )
                                ees[j] = ee
                                if j >= 2:
                                    emit_av(j - 2)
                            if n >= 2:
                                emit_av(n - 2)
                            emit_av(n - 1)
                        # normalize in SBUF off the PE critical path; both
                        # PSUM copies go first so the accumulator banks free
                        # up for the next row as fast as possible
                        uas = []
                        for par, aps in ((0, apair), (1, apodd)):
                            ua = normp.tile(
                                [HS + 1, QT], f32, tag="ua", name=f"ua_{qt}_{hp}_{par}"
                            )
                            nc.vector.tensor_copy(ua, aps)
                            uas.append(ua)
                        for par in range(2):
                            ua = uas[par]
                            rec = normp.tile([1, QT], f32, tag="rec", bufs=1)
                            nc.vector.reciprocal(rec, ua[HS : HS + 1, :])
                            bc = normp.tile([HS, QT], f32, tag="bc", bufs=1)
                            nc.gpsimd.partition_broadcast(bc, rec)
                            nc.vector.tensor_mul(
                                attnT[par * HS : (par + 1) * HS, hp, :],
                                ua[0:HS, :],
                                bc,
                            )
                    # LN1 + FFN1 of early token tiles ride the later
                    # attention rows as PE filler (keeps the HAM activity
                    # window warm); placed late enough that their RS chunk
                    # has landed before the engine queues reach them
                    if STAGE >= 4 and qt == 3:
                        ln1_tile(1)
                        ffn1_quarter(1)
                    # proj for this qt's 4 token tiles, then RS chunk qt
                    if STAGE < 3:
                        continue
                    for mtl in range(4):
                        mt = 4 * qt + mtl
                        prj = projp.tile([P, D], bf16, tag="prj")
                        for nh in range(D // QT):
                            pps = pssc.tile(
                                [P, QT], f32, tag="sc", name=f"pps_{mt}_{nh}"
                            )
                            for ko in range(2):
                                nc.tensor.matmul(
                                    pps,
                                    attnT[:, ko, mtl * P : (mtl + 1) * P],
                                    wp_sb[:, ko, nh * QT : (nh + 1) * QT],
                                    start=(ko == 0),
                                    stop=(ko == 1),
                                )
                            nc.vector.tensor_copy(prj[:, nh * QT : (nh + 1) * QT], pps)
                        nc.sync.dma_start(
                            rs_in[qt].ap()[mtl * P : (mtl + 1) * P, :], prj
                        )
                    nc.gpsimd.collective_compute(
                        "ReduceScatter",
                        mybir.AluOpType.add,
                        replica_groups=REPLICA_GROUPS,
                        ins=[rs_in[qt].ap().opt()],
                        outs=[rs_out[qt].ap().opt()],
                    )
                    # pipelined phase B behind the collective chunks
                    if STAGE >= 4:
                        if qt == 2:
                            ln1_tile(0)
                            ffn1_quarter(0)
                        elif qt == 3:
                            ffn2_half(0)
                            ln1_tile(2)
                            ffn1_quarter(2)
                            ln1_tile(3)
                            ffn1_quarter(3)
                            ffn2_half(1)

                if STAGE < 4:
                    dummy = ln1p.tile([P, D], f32, tag="tmp", name="dummy")
                    nc.vector.memset(dummy, 0.0)
                    for st in range(SL // P):
                        nc.sync.dma_start(out[st * P : (st + 1) * P, :], dummy)

            keep_cm.__exit__(None, None, None)

    nc.compile()
    return nc


_NC_CACHE = []


def _get_nc():
    if not _NC_CACHE:
        _NC_CACHE.append(build_bass())
    return _NC_CACHE[0]


def _token_blocks(r):
    """Global token rows (within a batch element) owned by rank r, as NCK
    blocks of 128: block ck covers rows [512*ck + 128*r, 512*ck + 128*r + 128)."""
    return [slice(QT * ck + P * r, QT * ck + P * r + P) for ck in range(NCK)]


def make_in_maps(x, wq, wk, wv, w_proj, b_proj, w1, b1, w2, b2, g1, be1, g2, be2):
    x = np.asarray(x, dtype=np.float32)
    bp = np.asarray(b_proj, dtype=np.float32)
    cat = lambda w, h0: np.ascontiguousarray(
        np.concatenate(
            [np.asarray(w[h0 + i], dtype=np.float32) for i in range(HPC)], axis=1
        )
    ).astype(bfnp)
    cvec_rows = [b_proj, g1, be1, g2, be2, b2]
    cvec = np.ascontiguousarray(
        np.stack([np.asarray(v, dtype=np.float32) for v in cvec_rows])
    )
    w1c = np.ascontiguousarray(np.asarray(w1, dtype=np.float32)).astype(bfnp)
    w2c = np.ascontiguousarray(np.asarray(w2, dtype=np.float32)).astype(bfnp)
    b1c = np.ascontiguousarray(np.asarray(b1, dtype=np.float32))
    wpc = np.asarray(w_proj, dtype=np.float32)
    xTs = [np.ascontiguousarray(x[g].T.astype(bfnp)) for g in range(B)]
    in_maps = []
    for c in range(NCORES):
        g, r = divmod(c, TP)
        h0 = HPC * r
        # b_proj is folded into the residual here instead of on device
        xs_blocks = np.concatenate(
            [x[g, blk] + bp for blk in _token_blocks(r)], axis=0
        )
        in_maps.append(
            {
                "xT": xTs[g],
                "xs": np.ascontiguousarray(xs_blocks),
                "wq2": cat(wq, h0),
                "wk2": cat(wk, h0),
                "wv4": cat(wv, h0),
                "wp": np.ascontiguousarray(
                    wpc[HPC * HS * r : HPC * HS * (r + 1)].astype(bfnp)
                ),
                "w1": w1c,
                "w2": w2c,
                "cvec": cvec,
                "b1d": b1c,
            }
        )
    return in_maps


def assemble(results):
    full = np.empty((B, S, D), dtype=np.float32)
    for c in range(NCORES):
        g, r = divmod(c, TP)
        o = results[c]["out"]
        for ck, blk in enumerate(_token_blocks(r)):
            full[g, blk] = o[ck * P : (ck + 1) * P]
    return full


def kernel(**inputs):
    nc = _get_nc()
    in_maps = make_in_maps(**inputs)
    res = run_bass_kernel_spmd(nc, in_maps, core_ids=list(range(NCORES)))
    return assemble(res.results)
